# revision 50
# baseline (speedup 1.0000x reference)
"""Trainium2 Bass kernel for a Mamba-1-style MixerBlock.

Reference computation (shapes: X[2,1024,1024], D=2048, N=16, K=4):
  Xn = LayerNorm(X) * g + b
  X_main = silu(conv_b + causal_depthwise_conv1d(Xn @ W_up1.T))
  pp = X_main @ W_ll.T + b_ll ; delta = softplus(pp[:, :D]); Bm, Cm = pp[:, D:D+N], pp[:, D+N:]
  a = exp(delta * A)  (A = -exp(A_log), [D,N])
  u = (a-1)/A * Bm * X_main        (per (b,l,d,n))
  h[t] = a[t] h[t-1] + u[t]        (scan over L per (b,d,n))
  y_ssm[t,d] = sum_n Cm[t,n] h[t,d,n]
  out = X + (y_ssm * silu(Xn @ W_up2.T)) @ W_down.T + b_down

Sharding: sequence-parallel over 8 cores (2 batches x 4 L-quarters of 256).
Each core redundantly recomputes a WARM-step scan warmup (decays are fast),
so the kernel is embarrassingly parallel - no collectives.

Per-core layout: channels on partitions, sequence on the free dim.
All matmuls run in fp16 (PE 1 cycle/row; fp32 PSUM accumulate); the SSM
elementwise chain runs in fp16 (DVE 2x for tensor_tensor; scan keeps an
fp32 internal state). The L-scan is a native tensor_tensor_scan chaining
n-segments per instruction (decay zeroed at segment starts encodes h=u).
Engine balance: scan on POOL, u split DVE/POOL, w/hci/tree/gate on DVE.
"""

import functools
import numpy as np

D_OUTER, D, N, K = 1024, 2048, 16, 4
B_SZ, L = 2, 1024
NCORES = 8
LO = 256            # own sequence steps per core
WARM = 24           # redundant scan warmup steps
LW = WARM + LO      # domain of X_main/delta/scan
LC = LW + K         # LayerNorm/mm1 domain (conv taps + even pad)
NT_D = D // 128     # 16 d-tiles
NT_K = D_OUTER // 128  # 8 k-tiles over d_outer
last_result = None
NG = 2              # n-groups for a/w/u/scan (pipelining granularity)
NH = N // NG        # n-values per group
# d-tiles whose hci/tree/gate run on POOL (engine balance; scan/u are
# DVE-only: the Pool engine does not implement TensorScalarPtr)
HC_DVE = {0, 5, 10, 15}


@functools.lru_cache(maxsize=2)
def _build_program(phases: str = "0ABCD"):
    import concourse.bass as bass
    import concourse.bacc as bacc
    import concourse.mybir as mybir
    import concourse.tile as tile
    from concourse.masks import make_identity

    f32 = mybir.dt.float32
    f16 = mybir.dt.float16
    AF = mybir.ActivationFunctionType
    OP = mybir.AluOpType

    # Steer the act-table-load pass: keep Exp and Ln only in their shared
    # set so phase C needs a single table load (ids/order preserved).
    import concourse.hw_specs as hw_specs
    if not getattr(bacc, "_act_tables_patched", False):
        _orig_gat = hw_specs.get_activation_tables

        def _gat(module_arch):
            tabs = _orig_gat(module_arch)
            AT = mybir.ActivationFunctionType
            for name, fns in tabs.items():
                if name != "natural_log_exp_and_others":
                    fns.discard(AT.Exp)
                    fns.discard(AT.Ln)
            return tabs

        bacc.get_activation_tables = _gat
        bacc._act_tables_patched = True

    nc = bacc.Bacc("TRN2", target_bir_lowering=False)

    # ---- DRAM I/O ----
    # Weights arrive pre-blocked so each per-d-tile stream is one contiguous
    # [128, contraction*128] read (2KB+ rows -> full DMA bandwidth).
    Xs_d = nc.dram_tensor("Xs", [LC, D_OUTER], f32, kind="ExternalInput")
    W1P_d = nc.dram_tensor("W1P", [NT_D * 128, NT_K * 128], f16,
                           kind="ExternalInput")
    W2P_d = nc.dram_tensor("W2P", [NT_D * 128, NT_K * 128], f16,
                           kind="ExternalInput")
    WllP_d = nc.dram_tensor("WllP", [NT_D * 128, NT_D * 128], f16,
                            kind="ExternalInput")
    WbcP_d = nc.dram_tensor("WbcP", [128, NT_D * 2 * N], f16,
                            kind="ExternalInput")
    WdP_d = nc.dram_tensor("WdP", [NT_K * 128, NT_D * 128], f16,
                           kind="ExternalInput")
    # packed per-d-tile constants: [128, NT_D, K+3+N]
    # (conv taps, conv bias, delta bias, gate bias, A row)
    CW = K + 3 + N
    cpk_d = nc.dram_tensor("cpk", [128, NT_D * CW], f32, kind="ExternalInput")
    # packed [2N, 2]: col 0 = b_ll[D:], col 1 = 1/A (B rows) or 1 (C rows)
    bvk_d = nc.dram_tensor("bvk", [2 * N, 2], f32, kind="ExternalInput")
    # packed [128, NT_K]: col e8 = b_down[e8*128:(e8+1)*128]
    bdown_d = nc.dram_tensor("bdown", [128, NT_K], f32, kind="ExternalInput")
    mask_d = nc.dram_tensor("mask", [1, LW], f32, kind="ExternalInput")
    Y_d = nc.dram_tensor("Y", [D_OUTER, LO], f32, kind="ExternalOutput")

    def bcast_n(t, nrep):
        # stride-0 broadcast of a [128, F] tile to [128, nrep, F]
        return bass.AP(tensor=t.tensor, offset=t.offset,
                       ap=[t.ap[0], [0, nrep], t.ap[1]])

    with tile.TileContext(nc) as tc:
        with (
            tc.tile_pool(name="const", bufs=1) as const,
            tc.tile_pool(name="persist", bufs=1) as persist,
            tc.tile_pool(name="work", bufs=2) as work,
            tc.tile_pool(name="big", bufs=2) as big,
            tc.tile_pool(name="bigwu", bufs=2) as bigwu,
            tc.tile_pool(name="red", bufs=1) as red,
            tc.tile_pool(name="wstream", bufs=2) as wstream,
            tc.tile_pool(name="psT", bufs=2, space="PSUM") as psT,
            tc.tile_pool(name="psB", bufs=1, space="PSUM") as psB,
            tc.tile_pool(name="psD", bufs=1, space="PSUM") as psD,
            tc.tile_pool(name="psA", bufs=4, space="PSUM") as psA,
        ):
            # ---- Phase 0 pool (row tiles processed one at a time) ----
            rows = [128, 128, LC - 256]
            p0_cm = tc.tile_pool(name="p0", bufs=2)
            p0 = p0_cm.__enter__()

            # ---- constants (packed DMAs) ----
            ident = const.tile([128, 128], f16, tag="ident")
            make_identity(nc, ident)
            eps_sb = const.tile([128, 1], f32, tag="eps")
            nc.vector.memset(eps_sb, 1e-5)

            cpk_sb = const.tile([128, NT_D, CW], f32, tag="cpk")
            nc.sync.dma_start(
                out=cpk_sb, in_=cpk_d.rearrange("p (dt f) -> p dt f", f=CW))
            convw_sb = [cpk_sb[:, dt, 0:K] for dt in range(NT_D)]
            cb2_sb = [cpk_sb[:, dt, K:K + 1] for dt in range(NT_D)]
            bd_sb = [cpk_sb[:, dt, K + 1:K + 2] for dt in range(NT_D)]
            c2_sb = [cpk_sb[:, dt, K + 2:K + 3] for dt in range(NT_D)]
            A_sb = [cpk_sb[:, dt, K + 3:K + 3 + N] for dt in range(NT_D)]

            bvk_sb = const.tile([2 * N, 2], f32, tag="bvk")
            nc.sync.dma_start(out=bvk_sb, in_=bvk_d[:, :])
            bbc_sb = bvk_sb[:, 0:1]
            invAv_sb = bvk_sb[:, 1:2]
            mask_sb = const.tile([2 * N, LW], f32, tag="mask")
            m_ap = mask_d[:, :]
            nc.sync.dma_start(
                out=mask_sb,
                in_=bass.AP(tensor=m_ap.tensor, offset=m_ap.offset,
                            ap=[[0, 2 * N], m_ap.ap[1]]))
            bdown_pk = const.tile([128, NT_K], f32, tag="bdn")
            nc.sync.dma_start(out=bdown_pk, in_=bdown_d[:, :])
            bdown_sb = [bdown_pk[:, e8:e8 + 1] for e8 in range(NT_K)]

            # ---- Phase 0: per row-tile: load, LayerNorm, transpose ----
            xhatT = []
            for kt in range(NT_K):
                xt = persist.tile([128, LC], f16, tag=f"xhT{kt}")
                xhatT.append(xt)
            dres_cm = tc.tile_pool(name="dres", bufs=1, space="DRAM")
            drp = dres_cm.__enter__()
            mu_d = drp.tile([3 * 128, 1], f32, tag="mu_d")
            sig_d = drp.tile([3 * 128, 1], f32, tag="sig_d")
            for i in range(3):
                r = rows[i]
                xr = p0.tile([128, D_OUTER], f32, tag="xr")
                # 4 chunked DMAs per row-tile to spread across DMA engines
                step = (r + 3) // 4
                for c0 in range(0, r, step):
                    c1 = min(c0 + step, r)
                    nc.sync.dma_start(
                        out=xr[c0:c1, :],
                        in_=Xs_d[i * 128 + c0:i * 128 + c1, :])
                # bn_stats free-dim max is 512: two subgroups then aggregate
                stats = work.tile([128, 2, 6], f32, tag="stats")
                for sg in range(2):
                    nc.vector.bn_stats(out=stats[:r, sg, :],
                                       in_=xr[:r, sg * 512:(sg + 1) * 512])
                mv = work.tile([128, 2], f32, tag="mv")
                nc.vector.bn_aggr(out=mv[:r, :], in_=stats[:r, :, :])
                sig = work.tile([128, 1], f32, tag="sig")
                nc.scalar.activation(out=sig[:r], in_=mv[:r, 1:2],
                                     func=AF.Sqrt, bias=eps_sb[:r, 0:1],
                                     scale=1.0)
                rsig = work.tile([128, 1], f32, tag="rsig")
                nc.vector.reciprocal(out=rsig[:r], in_=sig[:r])
                nmu = work.tile([128, 1], f32, tag="nmu")
                nc.vector.tensor_scalar(out=nmu[:r], in0=mv[:r, 0:1],
                                        scalar1=rsig[:r, 0:1], scalar2=-1.0,
                                        op0=OP.mult, op1=OP.mult)
                xh = p0.tile([128, D_OUTER], f16, tag="xh")
                nc.vector.tensor_scalar(out=xh[:r, :], in0=xr[:r, :],
                                        scalar1=rsig[:r, 0:1],
                                        scalar2=nmu[:r, 0:1],
                                        op0=OP.mult, op1=OP.add)
                # stage mu/sig to DRAM (read back broadcast for the residual)
                nc.sync.dma_start(out=mu_d[i * 128:i * 128 + r, :],
                                  in_=mv[:r, 0:1])
                nc.sync.dma_start(out=sig_d[i * 128:i * 128 + r, :],
                                  in_=sig[:r])
                for kt in range(NT_K):
                    cs = slice(kt * 128, (kt + 1) * 128)
                    pt = psT.tile([128, 128], f16, tag="tp")
                    nc.tensor.transpose(pt[:, :r], xh[:r, cs],
                                        ident[:r, :r])
                    nc.scalar.copy(out=xhatT[kt][:, i * 128:i * 128 + r],
                                   in_=pt[:, :r])
            mu_bc = persist.tile([128, LO], f32, tag="mu_bc")
            sig_bc = persist.tile([128, LO], f32, tag="sig_bc")
            own0 = WARM + K - 1
            for (dst, srcd) in ((mu_bc, mu_d), (sig_bc, sig_d)):
                s_ap = srcd[own0:own0 + LO, :]
                nc.sync.dma_start(
                    out=dst,
                    in_=bass.AP(tensor=s_ap.tensor, offset=s_ap.offset,
                                ap=[[0, 128], [1, LO]]))
            dres_cm.__exit__(None, None, None)
            p0_cm.__exit__(None, None, None)

            # ---- Phase A (+A2+B interleaved per d-tile) ----
            # mm1 + causal depthwise conv + silu -> X_main; gate mm2; and
            # the B/C projection accumulates incrementally so phase C can
            # start right after the last X_main tile.
            wbt = wstream.tile([128, NT_D, 2 * N], f16, tag="wbc")
            nc.sync.dma_start(
                out=wbt.rearrange("p kt e -> p (kt e)"),
                in_=WbcP_d[:, :])
            psbc = psB.tile([2 * N, LW], f32, tag="bc")
            X_main = []
            X_gate = []
            gate_silus = []
            for dt in range(NT_D if "A" in phases else 0):
                w1t = wstream.tile([128, NT_K, 128], f16, tag="w1")
                nc.sync.dma_start(
                    out=w1t.rearrange("p kt m -> p (kt m)"),
                    in_=W1P_d[dt * 128:(dt + 1) * 128, :])
                ps = psA.tile([128, LC], f32, tag="mm")
                for kt in range(NT_K):
                    nc.tensor.matmul(ps, w1t[:, kt, :],
                                     xhatT[kt],
                                     start=(kt == 0), stop=(kt == NT_K - 1))
                # depthwise conv: per-tap scaled copies on ACT (scale is the
                # per-channel tap weight), tap-sum via identity matmuls on PE
                pre16 = work.tile([128, K, LC], f16, tag="pre")
                for tap in range(K):
                    nc.scalar.activation(out=pre16[:, tap, :], in_=ps,
                                         func=AF.Identity,
                                         bias=0.0,
                                         scale=convw_sb[dt][:, tap:tap + 1])
                psC = psA.tile([128, LW], f32, tag="mm")
                for tap in range(K):
                    nc.tensor.matmul(psC, ident,
                                     pre16[:, tap, tap:tap + LW],
                                     start=(tap == 0), stop=(tap == K - 1))
                xm = persist.tile([128, LW], f16, tag=f"xm{dt}")
                nc.scalar.activation(out=xm, in_=psC, func=AF.Silu,
                                     bias=cb2_sb[dt][:, 0:1], scale=1.0)
                X_main.append(xm)
                # gate mm2 for this d-tile
                w2t = wstream.tile([128, NT_K, 128], f16, tag="w2")
                nc.sync.dma_start(
                    out=w2t.rearrange("p kt m -> p (kt m)"),
                    in_=W2P_d[dt * 128:(dt + 1) * 128, :])
                ps2 = psA.tile([128, LO], f32, tag="mm")
                for kt in range(NT_K):
                    nc.tensor.matmul(ps2, w2t[:, kt, :],
                                     xhatT[kt][:, WARM + K - 1:WARM + K - 1 + LO],
                                     start=(kt == 0), stop=(kt == NT_K - 1))
                xg = persist.tile([128, LO], f16, tag=f"xg{dt}")
                si = nc.scalar.activation(out=xg, in_=ps2, func=AF.Silu,
                                          bias=c2_sb[dt][:, 0:1], scale=1.0)
                gate_silus.append(si)
                X_gate.append(xg)
                # incremental B/C projection accumulate
                nc.tensor.matmul(psbc, wbt[:, dt, :], xm,
                                 start=(dt == 0), stop=(dt == NT_D - 1))

            # ---- Phase B: bias/scale/mask + partition-broadcast ----
            bc_raw = work.tile([2 * N, LW], f32, tag="bcraw")
            nc.scalar.activation(out=bc_raw, in_=psbc, func=AF.Identity,
                                 bias=bbc_sb[:, 0:1], scale=1.0)
            bci = work.tile([2 * N, LW], f16, tag="bci")
            nc.vector.scalar_tensor_tensor(out=bci, in0=bc_raw,
                                           scalar=invAv_sb[:, 0:1],
                                           in1=mask_sb, op0=OP.mult,
                                           op1=OP.mult)
            Bm_bcI = persist.tile([128, N, LW], f16, tag="BmbcI")
            Cm_bc = persist.tile([128, N, LO], f16, tag="Cmbc")
            with tc.tile_pool(name="dstage", bufs=1, space="DRAM") as dpool:
                bci_dram = dpool.tile([2 * N, LW], f16, tag="bcid")
                nc.sync.dma_start(out=bci_dram, in_=bci)
                for n in range(N):
                    src_b = bci_dram[n:n + 1, :]
                    nc.sync.dma_start(
                        out=Bm_bcI[:, n, :],
                        in_=bass.AP(tensor=src_b.tensor, offset=src_b.offset,
                                    ap=[[0, 128]] + src_b.ap[1:]))
                    src_c = bci_dram[N + n:N + n + 1, WARM:LW]
                    nc.sync.dma_start(
                        out=Cm_bc[:, n, :],
                        in_=bass.AP(tensor=src_c.tensor, offset=src_c.offset,
                                    ap=[[0, 128]] + src_c.ap[1:]))

            # residual precompute: xres = xhatT*sig + mu + b_down (per e8)
            xres = []
            for e8 in range(NT_K):
                xrec = work.tile([128, LO], f32, tag="xrec")
                nc.gpsimd.tensor_tensor(out=xrec,
                                        in0=xhatT[e8]
                                        [:, WARM + K - 1:WARM + K - 1 + LO],
                                        in1=sig_bc, op=OP.mult)
                xr2 = persist.tile([128, LO], f32, tag=f"xres{e8}")
                nc.vector.scalar_tensor_tensor(
                    out=xr2, in0=xrec, scalar=bdown_sb[e8][:, 0:1],
                    in1=mu_bc, op0=OP.add, op1=OP.add)
                xres.append(xr2)

            # ---- Phase C: per d-tile: delta, a, u, scan, y ----
            # (phase D partial accumulations interleave after dt 9 and 13)
            y_gated = []
            xy1 = []

            def emit_phase_d_part(d0, d1, first):
                # accumulate sum_dt WdT.yg for dt in [d0,d1) into xy1[e8]
                for e8 in range(NT_K):
                    wdt = wstream.tile([128, d1 - d0, 128], f16,
                                       tag="wd8" if d1 - d0 == 4 else "wd0")
                    nc.sync.dma_start(
                        out=wdt.rearrange("p dt m -> p (dt m)"),
                        in_=WdP_d[e8 * 128:(e8 + 1) * 128,
                                  d0 * 128:d1 * 128])
                    ps = psD.tile([128, LO], f32, tag="mm")
                    for i in range(d1 - d0):
                        nc.tensor.matmul(ps, wdt[:, i, :], y_gated[d0 + i],
                                         start=(i == 0), stop=(i == d1 - d0 - 1))
                    xy = work.tile([128, LO], f32, tag="xy")
                    nc.scalar.activation(out=xy, in_=ps, func=AF.Identity,
                                         bias=0.0, scale=1.0)
                    if first:
                        xy2 = persist.tile([128, LO], f32, tag=f"xy2_{e8}")
                        nc.gpsimd.tensor_tensor(out=xy2, in0=xy,
                                                in1=xres[e8], op=OP.add)
                        xy1.append(xy2)
                    else:
                        nc.gpsimd.tensor_tensor(out=xy1[e8], in0=xy,
                                                in1=xy1[e8], op=OP.add)

            for dt in range(NT_D):
                wllt = wstream.tile([128, NT_D, 128], f16, tag="wst")
                nc.sync.dma_start(
                    out=wllt.rearrange("p kt m -> p (kt m)"),
                    in_=WllP_d[dt * 128:(dt + 1) * 128, :])
                ps = psA.tile([128, LW], f32, tag="mm")
                for kt in range(NT_D):
                    nc.tensor.matmul(ps, wllt[:, kt, :],
                                     X_main[kt],
                                     start=(kt == 0), stop=(kt == NT_D - 1))
                # softplus(x) = ln(exp(x) + 1); exp & ln share one ACT table set
                e1 = work.tile([128, LW], f32, tag="e1")
                e1i = nc.scalar.activation(out=e1, in_=ps, func=AF.Exp,
                                           bias=bd_sb[dt][:, 0:1], scale=1.0)
                if dt == 0:
                    from concourse.tile_rust import add_dep_helper
                    for si in gate_silus:
                        add_dep_helper(e1i.ins, si.ins, False,
                                       "ACT table-set phase ordering")
                delta = work.tile([128, LW], f32, tag="delta")
                nc.scalar.activation(out=delta, in_=e1, func=AF.Ln,
                                     bias=1.0, scale=1.0)

                hc_eng = nc.vector if dt in HC_DVE else nc.gpsimd
                hci = red.tile([128, N, LO], f16, tag="hci")
                a_t = big.tile([128, N, LW], f16, tag="a")
                for n in range(N):
                    nc.scalar.activation(out=a_t[:, n, :], in_=delta,
                                         func=AF.Exp, bias=0.0,
                                         scale=A_sb[dt][:, n:n + 1])
                w_t = bigwu.tile([128, N, LW], f16, tag="w")
                nc.vector.tensor_tensor(
                    out=w_t, in0=bcast_n(X_main[dt], N),
                    in1=Bm_bcI[:, :, :], op=OP.mult)
                # u = (a-1)*w as TS(4x) + in-place TT(2x): beats one STT
                u_t = bigwu.tile([128, N, LW], f16, tag="u")
                nc.vector.tensor_scalar(
                    out=u_t.rearrange("p n l -> p (n l)"),
                    in0=a_t.rearrange("p n l -> p (n l)"),
                    scalar1=-1.0, scalar2=None, op0=OP.add)
                nc.vector.tensor_tensor(
                    out=u_t, in0=u_t, in1=w_t, op=OP.mult)
                # zero decay at each n-segment start: encodes h(start)=u
                nc.vector.memset(a_t[:, :, 0:1], 0.0)
                h_t = big.tile([128, N, LW], f16, tag="h")
                nc.vector.tensor_tensor_scan(
                    out=h_t.rearrange("p n l -> p (n l)"),
                    data0=a_t.rearrange("p n l -> p (n l)"),
                    data1=u_t.rearrange("p n l -> p (n l)"),
                    initial=0.0, op0=OP.mult, op1=OP.add)
                hc_eng.tensor_tensor(
                    out=hci,
                    in0=h_t[:, :, WARM:LW], in1=Cm_bc[:, :, :],
                    op=OP.mult)
                # reduce over n: pairwise tree, in place in the low half
                # of hci (fp16 is 2x on DVE)
                for lv in (2, 4, 8):
                    hc_eng.tensor_tensor(out=hci[:, 0:N // lv, :],
                                         in0=hci[:, 0:N // lv, :],
                                         in1=hci[:, N // lv:2 * N // lv, :],
                                         op=OP.add)
                ysum = red.tile([128, LO], f16, tag="ysum")
                hc_eng.tensor_tensor(out=ysum, in0=hci[:, 0, :],
                                     in1=hci[:, 1, :], op=OP.add)
                yg = persist.tile([128, LO], f16, tag=f"yg{dt}")
                hc_eng.tensor_tensor(out=yg, in0=ysum, in1=X_gate[dt],
                                     op=OP.mult)
                y_gated.append(yg)
                if dt == 9:
                    emit_phase_d_part(0, 8, first=True)
                elif dt == 13:
                    emit_phase_d_part(8, 12, first=False)

            # ---- Phase D (last 4 d-tiles): down projection + residual ----
            for e8 in range(NT_K):
                wdt = wstream.tile([128, 4, 128], f16, tag="wd8")
                nc.sync.dma_start(
                    out=wdt.rearrange("p dt m -> p (dt m)"),
                    in_=WdP_d[e8 * 128:(e8 + 1) * 128, 12 * 128:])
                ps = psD.tile([128, LO], f32, tag="mm")
                for i in range(4):
                    nc.tensor.matmul(ps, wdt[:, i, :],
                                     y_gated[12 + i],
                                     start=(i == 0), stop=(i == 3))
                osb = work.tile([128, LO], f32, tag="osb")
                nc.vector.tensor_tensor(out=osb, in0=ps, in1=xy1[e8],
                                        op=OP.add)
                nc.sync.dma_start(out=Y_d[e8 * 128:(e8 + 1) * 128, :], in_=osb)

    nc.compile()
    return nc


def kernel(X, ln_g, ln_b, W_up1, conv_w, conv_b, W_ll, b_ll, A_log, W_up2,
           W_down, b_down):
    from concourse.bass_utils import run_bass_kernel_spmd

    f = np.float32
    h = np.float16
    X = np.asarray(X, f)
    A = -np.exp(np.asarray(A_log, f))
    assert np.allclose(A, A[0:1, :]), "kernel assumes A rows identical"
    c1 = (np.asarray(W_up1, f) @ np.asarray(ln_b, f)).astype(f)
    c2 = (np.asarray(W_up2, f) @ np.asarray(ln_b, f)).astype(f)
    cw = np.asarray(conv_w, f)[:, 0, :]                      # [D, K]
    cb2 = (np.asarray(conv_b, f) + c1 * cw.sum(1)).astype(f)
    # packed per-d-tile constants [128, NT_D, K+3+N]
    CW = K + 3 + N
    bd_ll = np.asarray(b_ll, f)
    cpk = np.empty((128, D // 128, CW), f)
    for dt in range(D // 128):
        r = slice(dt * 128, (dt + 1) * 128)
        cpk[:, dt, 0:K] = cw[r]
        cpk[:, dt, K] = cb2[r]
        cpk[:, dt, K + 1] = bd_ll[:D][r]
        cpk[:, dt, K + 2] = c2[r]
        cpk[:, dt, K + 3:] = A[r]
    bvk = np.stack([bd_ll[D:],
                    np.concatenate([1.0 / A[0], np.ones(N, f)])], axis=1)
    def block(w, nt_out):
        # [C, E] -> [E//128*128, C] blocked: out[e_t*128+p, c_t*128+m]
        # = w[c_t*128+p, e_t*128+m]  (per-e-tile contiguous weight stream)
        C, E = w.shape
        return np.ascontiguousarray(
            w.reshape(C // 128, 128, E // 128, 128)
             .transpose(2, 1, 0, 3).reshape(E, C))

    W1T = (np.asarray(W_up1, f) * np.asarray(ln_g, f)[None, :]).T.astype(h)
    W2T = (np.asarray(W_up2, f) * np.asarray(ln_g, f)[None, :]).T.astype(h)
    WllT = np.asarray(W_ll, f).T.astype(h)
    WdT = np.asarray(W_down, f).T.astype(h)
    shared = {
        "W1P": block(W1T, D // 128),
        "W2P": block(W2T, D // 128),
        "WllP": block(WllT[:, :D], D // 128),
        "WbcP": np.ascontiguousarray(
            WllT[:, D:].reshape(D // 128, 128, 2 * N)
            .transpose(1, 0, 2).reshape(128, -1)),
        "WdP": block(WdT, D_OUTER // 128),
        "cpk": np.ascontiguousarray(cpk.reshape(128, -1)),
        "bvk": np.ascontiguousarray(bvk.astype(f)),
        "bdown": np.ascontiguousarray(
            np.asarray(b_down, f).reshape(D_OUTER // 128, 128).T),
    }
    in_maps = []
    for c in range(NCORES):
        b, q = divmod(c, 4)
        l0 = q * LO
        lo_ext = l0 - (WARM + K - 1)
        xs = np.zeros((LC, D_OUTER), f)
        src0 = max(0, lo_ext)
        hi = min(l0 + LO + 1, L)
        xs[src0 - lo_ext:src0 - lo_ext + (hi - src0), :] = X[b, src0:hi, :]
        mask = np.ones((1, LW), f)
        if q == 0:
            mask[0, :WARM] = 0.0
        in_maps.append({"Xs": xs, "mask": mask, **shared})

    nc = _build_program()
    res = run_bass_kernel_spmd(nc, in_maps, core_ids=list(range(NCORES)))
    global last_result
    last_result = res

    out = np.empty((B_SZ, L, D_OUTER), f)
    for c in range(NCORES):
        b, q = divmod(c, 4)
        out[b, q * LO:(q + 1) * LO, :] = res.results[c]["Y"].T
    return out


# revision 52
# speedup vs baseline: 1.0486x; 1.0486x over previous
"""Trainium2 Bass kernel for a Mamba-1-style MixerBlock.

Reference computation (shapes: X[2,1024,1024], D=2048, N=16, K=4):
  Xn = LayerNorm(X) * g + b
  X_main = silu(conv_b + causal_depthwise_conv1d(Xn @ W_up1.T))
  pp = X_main @ W_ll.T + b_ll ; delta = softplus(pp[:, :D]); Bm, Cm = pp[:, D:D+N], pp[:, D+N:]
  a = exp(delta * A)  (A = -exp(A_log), [D,N])
  u = (a-1)/A * Bm * X_main        (per (b,l,d,n))
  h[t] = a[t] h[t-1] + u[t]        (scan over L per (b,d,n))
  y_ssm[t,d] = sum_n Cm[t,n] h[t,d,n]
  out = X + (y_ssm * silu(Xn @ W_up2.T)) @ W_down.T + b_down

Sharding: sequence-parallel over 8 cores (2 batches x 4 L-quarters of 256).
Each core redundantly recomputes a WARM-step scan warmup (decays are fast),
so the kernel is embarrassingly parallel - no collectives.

Per-core layout: channels on partitions, sequence on the free dim.
All matmuls run in fp16 (PE 1 cycle/row; fp32 PSUM accumulate); the SSM
elementwise chain runs in fp16 (DVE 2x for tensor_tensor; scan keeps an
fp32 internal state). The L-scan is a native tensor_tensor_scan chaining
n-segments per instruction (decay zeroed at segment starts encodes h=u).
Engine balance: scan on POOL, u split DVE/POOL, w/hci/tree/gate on DVE.
"""

import functools
import numpy as np

D_OUTER, D, N, K = 1024, 2048, 16, 4
B_SZ, L = 2, 1024
NCORES = 8
LO = 256            # own sequence steps per core
WARM = 24           # redundant scan warmup steps
LW = WARM + LO      # domain of X_main/delta/scan
LC = LW + K         # LayerNorm/mm1 domain (conv taps + even pad)
NT_D = D // 128     # 16 d-tiles
NT_K = D_OUTER // 128  # 8 k-tiles over d_outer
last_result = None
NG = 2              # n-groups for a/w/u/scan (pipelining granularity)
NH = N // NG        # n-values per group
# d-tiles whose hci/tree/gate run on POOL (engine balance; scan/u are
# DVE-only: the Pool engine does not implement TensorScalarPtr)
HC_DVE = {0, 5, 10, 15}


@functools.lru_cache(maxsize=2)
def _build_program(phases: str = "0ABCD"):
    import concourse.bass as bass
    import concourse.bacc as bacc
    import concourse.mybir as mybir
    import concourse.tile as tile
    from concourse.masks import make_identity

    f32 = mybir.dt.float32
    f16 = mybir.dt.float16
    AF = mybir.ActivationFunctionType
    OP = mybir.AluOpType

    # Steer the act-table-load pass: keep Exp and Ln only in their shared
    # set so phase C needs a single table load (ids/order preserved).
    import concourse.hw_specs as hw_specs
    if not getattr(bacc, "_act_tables_patched", False):
        _orig_gat = hw_specs.get_activation_tables

        def _gat(module_arch):
            tabs = _orig_gat(module_arch)
            AT = mybir.ActivationFunctionType
            for name, fns in tabs.items():
                if name != "natural_log_exp_and_others":
                    fns.discard(AT.Exp)
                    fns.discard(AT.Ln)
            return tabs

        bacc.get_activation_tables = _gat
        bacc._act_tables_patched = True

    nc = bacc.Bacc("TRN2", target_bir_lowering=False)

    # ---- DRAM I/O ----
    # Weights arrive pre-blocked so each per-d-tile stream is one contiguous
    # [128, contraction*128] read (2KB+ rows -> full DMA bandwidth).
    Xs_d = nc.dram_tensor("Xs", [LC, D_OUTER], f32, kind="ExternalInput")
    W1P_d = nc.dram_tensor("W1P", [NT_D * 128, NT_K * 128], f16,
                           kind="ExternalInput")
    W2P_d = nc.dram_tensor("W2P", [NT_D * 128, NT_K * 128], f16,
                           kind="ExternalInput")
    WllP_d = nc.dram_tensor("WllP", [NT_D * 128, NT_D * 128], f16,
                            kind="ExternalInput")
    WbcP_d = nc.dram_tensor("WbcP", [128, NT_D * 2 * N], f16,
                            kind="ExternalInput")
    WdP_d = nc.dram_tensor("WdP", [NT_K * 128, NT_D * 128], f16,
                           kind="ExternalInput")
    # packed per-d-tile constants: [128, NT_D, K+3+N]
    # (conv taps, conv bias, delta bias, gate bias, A row)
    CW = K + 3 + N
    cpk_d = nc.dram_tensor("cpk", [128, NT_D * CW], f32, kind="ExternalInput")
    # packed [2N, 2]: col 0 = b_ll[D:], col 1 = 1/A (B rows) or 1 (C rows)
    bvk_d = nc.dram_tensor("bvk", [2 * N, 2], f32, kind="ExternalInput")
    # packed [128, NT_K]: col e8 = b_down[e8*128:(e8+1)*128]
    bdown_d = nc.dram_tensor("bdown", [128, NT_K], f32, kind="ExternalInput")
    mask_d = nc.dram_tensor("mask", [1, LW], f32, kind="ExternalInput")
    Y_d = nc.dram_tensor("Y", [D_OUTER, LO], f32, kind="ExternalOutput")

    def bcast_n(t, nrep):
        # stride-0 broadcast of a [128, F] tile to [128, nrep, F]
        return bass.AP(tensor=t.tensor, offset=t.offset,
                       ap=[t.ap[0], [0, nrep], t.ap[1]])

    with tile.TileContext(nc) as tc:
        with (
            tc.tile_pool(name="const", bufs=1) as const,
            tc.tile_pool(name="persist", bufs=1) as persist,
            tc.tile_pool(name="work", bufs=2) as work,
            tc.tile_pool(name="big", bufs=2) as big,
            tc.tile_pool(name="bigwu", bufs=2) as bigwu,
            tc.tile_pool(name="red", bufs=2) as red,
            tc.tile_pool(name="wstream", bufs=2) as wstream,
            tc.tile_pool(name="psT", bufs=2, space="PSUM") as psT,
            tc.tile_pool(name="psB", bufs=1, space="PSUM") as psB,
            tc.tile_pool(name="psD", bufs=1, space="PSUM") as psD,
            tc.tile_pool(name="psA", bufs=4, space="PSUM") as psA,
        ):
            # ---- Phase 0 pool (row tiles processed one at a time) ----
            rows = [128, 128, LC - 256]
            p0_cm = tc.tile_pool(name="p0", bufs=2)
            p0 = p0_cm.__enter__()

            # ---- constants (packed DMAs) ----
            ident = const.tile([128, 128], f16, tag="ident")
            make_identity(nc, ident)
            eps_sb = const.tile([128, 1], f32, tag="eps")
            nc.vector.memset(eps_sb, 1e-5)

            cpk_sb = const.tile([128, NT_D, CW], f32, tag="cpk")
            nc.sync.dma_start(
                out=cpk_sb, in_=cpk_d.rearrange("p (dt f) -> p dt f", f=CW))
            convw_sb = [cpk_sb[:, dt, 0:K] for dt in range(NT_D)]
            cb2_sb = [cpk_sb[:, dt, K:K + 1] for dt in range(NT_D)]
            bd_sb = [cpk_sb[:, dt, K + 1:K + 2] for dt in range(NT_D)]
            c2_sb = [cpk_sb[:, dt, K + 2:K + 3] for dt in range(NT_D)]
            A_sb = [cpk_sb[:, dt, K + 3:K + 3 + N] for dt in range(NT_D)]

            bvk_sb = const.tile([2 * N, 2], f32, tag="bvk")
            nc.sync.dma_start(out=bvk_sb, in_=bvk_d[:, :])
            bbc_sb = bvk_sb[:, 0:1]
            invAv_sb = bvk_sb[:, 1:2]
            mask_sb = const.tile([2 * N, LW], f32, tag="mask")
            m_ap = mask_d[:, :]
            nc.sync.dma_start(
                out=mask_sb,
                in_=bass.AP(tensor=m_ap.tensor, offset=m_ap.offset,
                            ap=[[0, 2 * N], m_ap.ap[1]]))
            bdown_pk = const.tile([128, NT_K], f32, tag="bdn")
            nc.sync.dma_start(out=bdown_pk, in_=bdown_d[:, :])
            bdown_sb = [bdown_pk[:, e8:e8 + 1] for e8 in range(NT_K)]

            # ---- Phase 0: per row-tile: load, LayerNorm, transpose ----
            xhatT = []
            for kt in range(NT_K):
                xt = persist.tile([128, LC], f16, tag=f"xhT{kt}")
                xhatT.append(xt)
            dres_cm = tc.tile_pool(name="dres", bufs=1, space="DRAM")
            drp = dres_cm.__enter__()
            mu_d = drp.tile([3 * 128, 1], f32, tag="mu_d")
            sig_d = drp.tile([3 * 128, 1], f32, tag="sig_d")
            for i in range(3):
                r = rows[i]
                xr = p0.tile([128, D_OUTER], f32, tag="xr")
                # 4 chunked DMAs per row-tile to spread across DMA engines
                step = (r + 3) // 4
                for c0 in range(0, r, step):
                    c1 = min(c0 + step, r)
                    nc.sync.dma_start(
                        out=xr[c0:c1, :],
                        in_=Xs_d[i * 128 + c0:i * 128 + c1, :])
                # bn_stats free-dim max is 512: two subgroups then aggregate
                stats = work.tile([128, 2, 6], f32, tag="stats")
                for sg in range(2):
                    nc.vector.bn_stats(out=stats[:r, sg, :],
                                       in_=xr[:r, sg * 512:(sg + 1) * 512])
                mv = work.tile([128, 2], f32, tag="mv")
                nc.vector.bn_aggr(out=mv[:r, :], in_=stats[:r, :, :])
                sig = work.tile([128, 1], f32, tag="sig")
                nc.scalar.activation(out=sig[:r], in_=mv[:r, 1:2],
                                     func=AF.Sqrt, bias=eps_sb[:r, 0:1],
                                     scale=1.0)
                rsig = work.tile([128, 1], f32, tag="rsig")
                nc.vector.reciprocal(out=rsig[:r], in_=sig[:r])
                nmu = work.tile([128, 1], f32, tag="nmu")
                nc.vector.tensor_scalar(out=nmu[:r], in0=mv[:r, 0:1],
                                        scalar1=rsig[:r, 0:1], scalar2=-1.0,
                                        op0=OP.mult, op1=OP.mult)
                xh = p0.tile([128, D_OUTER], f16, tag="xh")
                nc.vector.tensor_scalar(out=xh[:r, :], in0=xr[:r, :],
                                        scalar1=rsig[:r, 0:1],
                                        scalar2=nmu[:r, 0:1],
                                        op0=OP.mult, op1=OP.add)
                # stage mu/sig to DRAM (read back broadcast for the residual)
                nc.sync.dma_start(out=mu_d[i * 128:i * 128 + r, :],
                                  in_=mv[:r, 0:1])
                nc.sync.dma_start(out=sig_d[i * 128:i * 128 + r, :],
                                  in_=sig[:r])
                for kt in range(NT_K):
                    cs = slice(kt * 128, (kt + 1) * 128)
                    pt = psT.tile([128, 128], f16, tag="tp")
                    nc.tensor.transpose(pt[:, :r], xh[:r, cs],
                                        ident[:r, :r])
                    nc.scalar.copy(out=xhatT[kt][:, i * 128:i * 128 + r],
                                   in_=pt[:, :r])
            mu_bc = persist.tile([128, LO], f32, tag="mu_bc")
            sig_bc = persist.tile([128, LO], f32, tag="sig_bc")
            own0 = WARM + K - 1
            for (dst, srcd) in ((mu_bc, mu_d), (sig_bc, sig_d)):
                s_ap = srcd[own0:own0 + LO, :]
                nc.sync.dma_start(
                    out=dst,
                    in_=bass.AP(tensor=s_ap.tensor, offset=s_ap.offset,
                                ap=[[0, 128], [1, LO]]))
            dres_cm.__exit__(None, None, None)
            p0_cm.__exit__(None, None, None)

            # ---- Phase A (+A2+B interleaved per d-tile) ----
            # mm1 + causal depthwise conv + silu -> X_main; gate mm2; and
            # the B/C projection accumulates incrementally so phase C can
            # start right after the last X_main tile.
            wbt = wstream.tile([128, NT_D, 2 * N], f16, tag="wbc")
            nc.sync.dma_start(
                out=wbt.rearrange("p kt e -> p (kt e)"),
                in_=WbcP_d[:, :])
            psbc = psB.tile([2 * N, LW], f32, tag="bc")
            X_main = []
            X_gate = []
            gate_silus = []
            for dt in range(NT_D if "A" in phases else 0):
                w1t = wstream.tile([128, NT_K, 128], f16, tag="w1")
                nc.sync.dma_start(
                    out=w1t.rearrange("p kt m -> p (kt m)"),
                    in_=W1P_d[dt * 128:(dt + 1) * 128, :])
                ps = psA.tile([128, LC], f32, tag="mm")
                for kt in range(NT_K):
                    nc.tensor.matmul(ps, w1t[:, kt, :],
                                     xhatT[kt],
                                     start=(kt == 0), stop=(kt == NT_K - 1))
                # depthwise conv: per-tap scaled copies on ACT (scale is the
                # per-channel tap weight), tap-sum via identity matmuls on PE
                pre16 = work.tile([128, K, LC], f16, tag="pre")
                for tap in range(K):
                    nc.scalar.activation(out=pre16[:, tap, :], in_=ps,
                                         func=AF.Identity,
                                         bias=0.0,
                                         scale=convw_sb[dt][:, tap:tap + 1])
                psC = psA.tile([128, LW], f32, tag="mm")
                for tap in range(K):
                    nc.tensor.matmul(psC, ident,
                                     pre16[:, tap, tap:tap + LW],
                                     start=(tap == 0), stop=(tap == K - 1))
                xm = persist.tile([128, LW], f16, tag=f"xm{dt}")
                nc.scalar.activation(out=xm, in_=psC, func=AF.Silu,
                                     bias=cb2_sb[dt][:, 0:1], scale=1.0)
                X_main.append(xm)
                # gate mm2 for this d-tile
                w2t = wstream.tile([128, NT_K, 128], f16, tag="w2")
                nc.sync.dma_start(
                    out=w2t.rearrange("p kt m -> p (kt m)"),
                    in_=W2P_d[dt * 128:(dt + 1) * 128, :])
                ps2 = psA.tile([128, LO], f32, tag="mm")
                for kt in range(NT_K):
                    nc.tensor.matmul(ps2, w2t[:, kt, :],
                                     xhatT[kt][:, WARM + K - 1:WARM + K - 1 + LO],
                                     start=(kt == 0), stop=(kt == NT_K - 1))
                xg = persist.tile([128, LO], f16, tag=f"xg{dt}")
                si = nc.scalar.activation(out=xg, in_=ps2, func=AF.Silu,
                                          bias=c2_sb[dt][:, 0:1], scale=1.0)
                gate_silus.append(si)
                X_gate.append(xg)
                # incremental B/C projection accumulate
                nc.tensor.matmul(psbc, wbt[:, dt, :], xm,
                                 start=(dt == 0), stop=(dt == NT_D - 1))

            # ---- Phase B: bias/scale/mask + partition-broadcast ----
            bc_raw = work.tile([2 * N, LW], f32, tag="bcraw")
            nc.scalar.activation(out=bc_raw, in_=psbc, func=AF.Identity,
                                 bias=bbc_sb[:, 0:1], scale=1.0)
            bci = work.tile([2 * N, LW], f16, tag="bci")
            nc.vector.scalar_tensor_tensor(out=bci, in0=bc_raw,
                                           scalar=invAv_sb[:, 0:1],
                                           in1=mask_sb, op0=OP.mult,
                                           op1=OP.mult)
            Bm_bcI = persist.tile([128, N, LW], f16, tag="BmbcI")
            Cm_bc = persist.tile([128, N, LO], f16, tag="Cmbc")
            with tc.tile_pool(name="dstage", bufs=1, space="DRAM") as dpool:
                bci_dram = dpool.tile([2 * N, LW], f16, tag="bcid")
                nc.sync.dma_start(out=bci_dram, in_=bci)
                for n in range(N):
                    src_b = bci_dram[n:n + 1, :]
                    nc.sync.dma_start(
                        out=Bm_bcI[:, n, :],
                        in_=bass.AP(tensor=src_b.tensor, offset=src_b.offset,
                                    ap=[[0, 128]] + src_b.ap[1:]))
                    src_c = bci_dram[N + n:N + n + 1, WARM:LW]
                    nc.sync.dma_start(
                        out=Cm_bc[:, n, :],
                        in_=bass.AP(tensor=src_c.tensor, offset=src_c.offset,
                                    ap=[[0, 128]] + src_c.ap[1:]))

            # residual precompute: xres = xhatT*sig + mu + b_down (per e8)
            xres = []
            for e8 in range(NT_K):
                xrec = work.tile([128, LO], f32, tag="xrec")
                nc.gpsimd.tensor_tensor(out=xrec,
                                        in0=xhatT[e8]
                                        [:, WARM + K - 1:WARM + K - 1 + LO],
                                        in1=sig_bc, op=OP.mult)
                xr2 = persist.tile([128, LO], f32, tag=f"xres{e8}")
                nc.vector.scalar_tensor_tensor(
                    out=xr2, in0=xrec, scalar=bdown_sb[e8][:, 0:1],
                    in1=mu_bc, op0=OP.add, op1=OP.add)
                xres.append(xr2)

            # ---- Phase C: per d-tile: delta, a, u, scan, y ----
            # (phase D partial accumulations interleave after dt 9 and 13)
            y_gated = []
            xy1 = []

            def emit_phase_d_part(d0, d1, first):
                # accumulate sum_dt WdT.yg for dt in [d0,d1) into xy1[e8]
                for e8 in range(NT_K):
                    wdt = wstream.tile([128, d1 - d0, 128], f16,
                                       tag="wd8" if d1 - d0 == 4 else "wd0")
                    nc.sync.dma_start(
                        out=wdt.rearrange("p dt m -> p (dt m)"),
                        in_=WdP_d[e8 * 128:(e8 + 1) * 128,
                                  d0 * 128:d1 * 128])
                    ps = psD.tile([128, LO], f32, tag="mm")
                    for i in range(d1 - d0):
                        nc.tensor.matmul(ps, wdt[:, i, :], y_gated[d0 + i],
                                         start=(i == 0), stop=(i == d1 - d0 - 1))
                    xy = work.tile([128, LO], f32, tag="xy")
                    nc.scalar.activation(out=xy, in_=ps, func=AF.Identity,
                                         bias=0.0, scale=1.0)
                    if first:
                        xy2 = persist.tile([128, LO], f32, tag=f"xy2_{e8}")
                        nc.gpsimd.tensor_tensor(out=xy2, in0=xy,
                                                in1=xres[e8], op=OP.add)
                        xy1.append(xy2)
                    else:
                        nc.gpsimd.tensor_tensor(out=xy1[e8], in0=xy,
                                                in1=xy1[e8], op=OP.add)

            for dt in range(NT_D):
                wllt = wstream.tile([128, NT_D, 128], f16, tag="wst")
                nc.sync.dma_start(
                    out=wllt.rearrange("p kt m -> p (kt m)"),
                    in_=WllP_d[dt * 128:(dt + 1) * 128, :])
                ps = psA.tile([128, LW], f32, tag="mm")
                for kt in range(NT_D):
                    nc.tensor.matmul(ps, wllt[:, kt, :],
                                     X_main[kt],
                                     start=(kt == 0), stop=(kt == NT_D - 1))
                # softplus(x) = ln(exp(x) + 1); exp & ln share one ACT table set
                e1 = work.tile([128, LW], f32, tag="e1")
                e1i = nc.scalar.activation(out=e1, in_=ps, func=AF.Exp,
                                           bias=bd_sb[dt][:, 0:1], scale=1.0)
                if dt == 0:
                    from concourse.tile_rust import add_dep_helper
                    for si in gate_silus:
                        add_dep_helper(e1i.ins, si.ins, False,
                                       "ACT table-set phase ordering")
                delta = work.tile([128, LW], f32, tag="delta")
                nc.scalar.activation(out=delta, in_=e1, func=AF.Ln,
                                     bias=1.0, scale=1.0)

                hc_eng = nc.vector if dt in HC_DVE else nc.gpsimd
                hci = red.tile([128, N, LO], f16, tag="hci")
                a_t = big.tile([128, N, LW], f16, tag="a")
                for n in range(N):
                    nc.scalar.activation(out=a_t[:, n, :], in_=delta,
                                         func=AF.Exp, bias=0.0,
                                         scale=A_sb[dt][:, n:n + 1])
                w_t = bigwu.tile([128, N, LW], f16, tag="w")
                nc.vector.tensor_tensor(
                    out=w_t, in0=bcast_n(X_main[dt], N),
                    in1=Bm_bcI[:, :, :], op=OP.mult)
                # u = (a-1)*w as TS(4x) + in-place TT(2x): beats one STT
                u_t = bigwu.tile([128, N, LW], f16, tag="u")
                nc.vector.tensor_scalar(
                    out=u_t.rearrange("p n l -> p (n l)"),
                    in0=a_t.rearrange("p n l -> p (n l)"),
                    scalar1=-1.0, scalar2=None, op0=OP.add)
                nc.vector.tensor_tensor(
                    out=u_t, in0=u_t, in1=w_t, op=OP.mult)
                # zero decay at each n-segment start: encodes h(start)=u
                nc.vector.memset(a_t[:, :, 0:1], 0.0)
                # scan writes over w_t (dead once u is formed)
                h_t = w_t
                nc.vector.tensor_tensor_scan(
                    out=h_t.rearrange("p n l -> p (n l)"),
                    data0=a_t.rearrange("p n l -> p (n l)"),
                    data1=u_t.rearrange("p n l -> p (n l)"),
                    initial=0.0, op0=OP.mult, op1=OP.add)
                hc_eng.tensor_tensor(
                    out=hci,
                    in0=h_t[:, :, WARM:LW], in1=Cm_bc[:, :, :],
                    op=OP.mult)
                # reduce over n: pairwise tree, in place in the low half
                # of hci (fp16 is 2x on DVE)
                for lv in (2, 4, 8):
                    hc_eng.tensor_tensor(out=hci[:, 0:N // lv, :],
                                         in0=hci[:, 0:N // lv, :],
                                         in1=hci[:, N // lv:2 * N // lv, :],
                                         op=OP.add)
                ysum = red.tile([128, LO], f16, tag="ysum")
                hc_eng.tensor_tensor(out=ysum, in0=hci[:, 0, :],
                                     in1=hci[:, 1, :], op=OP.add)
                yg = persist.tile([128, LO], f16, tag=f"yg{dt}")
                hc_eng.tensor_tensor(out=yg, in0=ysum, in1=X_gate[dt],
                                     op=OP.mult)
                y_gated.append(yg)
                if dt == 9:
                    emit_phase_d_part(0, 8, first=True)
                elif dt == 13:
                    emit_phase_d_part(8, 12, first=False)

            # ---- Phase D (last 4 d-tiles): down projection + residual ----
            for e8 in range(NT_K):
                wdt = wstream.tile([128, 4, 128], f16, tag="wd8")
                nc.sync.dma_start(
                    out=wdt.rearrange("p dt m -> p (dt m)"),
                    in_=WdP_d[e8 * 128:(e8 + 1) * 128, 12 * 128:])
                ps = psD.tile([128, LO], f32, tag="mm")
                for i in range(4):
                    nc.tensor.matmul(ps, wdt[:, i, :],
                                     y_gated[12 + i],
                                     start=(i == 0), stop=(i == 3))
                osb = work.tile([128, LO], f32, tag="osb")
                nc.vector.tensor_tensor(out=osb, in0=ps, in1=xy1[e8],
                                        op=OP.add)
                nc.sync.dma_start(out=Y_d[e8 * 128:(e8 + 1) * 128, :], in_=osb)

    nc.compile()
    return nc


def kernel(X, ln_g, ln_b, W_up1, conv_w, conv_b, W_ll, b_ll, A_log, W_up2,
           W_down, b_down):
    from concourse.bass_utils import run_bass_kernel_spmd

    f = np.float32
    h = np.float16
    X = np.asarray(X, f)
    A = -np.exp(np.asarray(A_log, f))
    assert np.allclose(A, A[0:1, :]), "kernel assumes A rows identical"
    c1 = (np.asarray(W_up1, f) @ np.asarray(ln_b, f)).astype(f)
    c2 = (np.asarray(W_up2, f) @ np.asarray(ln_b, f)).astype(f)
    cw = np.asarray(conv_w, f)[:, 0, :]                      # [D, K]
    cb2 = (np.asarray(conv_b, f) + c1 * cw.sum(1)).astype(f)
    # packed per-d-tile constants [128, NT_D, K+3+N]
    CW = K + 3 + N
    bd_ll = np.asarray(b_ll, f)
    cpk = np.empty((128, D // 128, CW), f)
    for dt in range(D // 128):
        r = slice(dt * 128, (dt + 1) * 128)
        cpk[:, dt, 0:K] = cw[r]
        cpk[:, dt, K] = cb2[r]
        cpk[:, dt, K + 1] = bd_ll[:D][r]
        cpk[:, dt, K + 2] = c2[r]
        cpk[:, dt, K + 3:] = A[r]
    bvk = np.stack([bd_ll[D:],
                    np.concatenate([1.0 / A[0], np.ones(N, f)])], axis=1)
    def block(w, nt_out):
        # [C, E] -> [E//128*128, C] blocked: out[e_t*128+p, c_t*128+m]
        # = w[c_t*128+p, e_t*128+m]  (per-e-tile contiguous weight stream)
        C, E = w.shape
        return np.ascontiguousarray(
            w.reshape(C // 128, 128, E // 128, 128)
             .transpose(2, 1, 0, 3).reshape(E, C))

    W1T = (np.asarray(W_up1, f) * np.asarray(ln_g, f)[None, :]).T.astype(h)
    W2T = (np.asarray(W_up2, f) * np.asarray(ln_g, f)[None, :]).T.astype(h)
    WllT = np.asarray(W_ll, f).T.astype(h)
    WdT = np.asarray(W_down, f).T.astype(h)
    shared = {
        "W1P": block(W1T, D // 128),
        "W2P": block(W2T, D // 128),
        "WllP": block(WllT[:, :D], D // 128),
        "WbcP": np.ascontiguousarray(
            WllT[:, D:].reshape(D // 128, 128, 2 * N)
            .transpose(1, 0, 2).reshape(128, -1)),
        "WdP": block(WdT, D_OUTER // 128),
        "cpk": np.ascontiguousarray(cpk.reshape(128, -1)),
        "bvk": np.ascontiguousarray(bvk.astype(f)),
        "bdown": np.ascontiguousarray(
            np.asarray(b_down, f).reshape(D_OUTER // 128, 128).T),
    }
    in_maps = []
    for c in range(NCORES):
        b, q = divmod(c, 4)
        l0 = q * LO
        lo_ext = l0 - (WARM + K - 1)
        xs = np.zeros((LC, D_OUTER), f)
        src0 = max(0, lo_ext)
        hi = min(l0 + LO + 1, L)
        xs[src0 - lo_ext:src0 - lo_ext + (hi - src0), :] = X[b, src0:hi, :]
        mask = np.ones((1, LW), f)
        if q == 0:
            mask[0, :WARM] = 0.0
        in_maps.append({"Xs": xs, "mask": mask, **shared})

    nc = _build_program()
    res = run_bass_kernel_spmd(nc, in_maps, core_ids=list(range(NCORES)))
    global last_result
    last_result = res

    out = np.empty((B_SZ, L, D_OUTER), f)
    for c in range(NCORES):
        b, q = divmod(c, 4)
        out[b, q * LO:(q + 1) * LO, :] = res.results[c]["Y"].T
    return out


# revision 53
# speedup vs baseline: 1.1545x; 1.1010x over previous
"""Trainium2 Bass kernel for a Mamba-1-style MixerBlock.

Reference computation (shapes: X[2,1024,1024], D=2048, N=16, K=4):
  Xn = LayerNorm(X) * g + b
  X_main = silu(conv_b + causal_depthwise_conv1d(Xn @ W_up1.T))
  pp = X_main @ W_ll.T + b_ll ; delta = softplus(pp[:, :D]); Bm, Cm = pp[:, D:D+N], pp[:, D+N:]
  a = exp(delta * A)  (A = -exp(A_log), [D,N])
  u = (a-1)/A * Bm * X_main        (per (b,l,d,n))
  h[t] = a[t] h[t-1] + u[t]        (scan over L per (b,d,n))
  y_ssm[t,d] = sum_n Cm[t,n] h[t,d,n]
  out = X + (y_ssm * silu(Xn @ W_up2.T)) @ W_down.T + b_down

Sharding: sequence-parallel over 8 cores (2 batches x 4 L-quarters of 256).
Each core redundantly recomputes a WARM-step scan warmup (decays are fast),
so the kernel is embarrassingly parallel - no collectives.

Per-core layout: channels on partitions, sequence on the free dim.
All matmuls run in fp16 (PE 1 cycle/row; fp32 PSUM accumulate); the SSM
elementwise chain runs in fp16 (DVE 2x for tensor_tensor; scan keeps an
fp32 internal state). The L-scan is a native tensor_tensor_scan chaining
n-segments per instruction (decay zeroed at segment starts encodes h=u).
Engine balance: scan on POOL, u split DVE/POOL, w/hci/tree/gate on DVE.
"""

import functools
import numpy as np

D_OUTER, D, N, K = 1024, 2048, 16, 4
B_SZ, L = 2, 1024
NCORES = 8
LO = 256            # own sequence steps per core
WARM = 24           # redundant scan warmup steps
LW = WARM + LO      # domain of X_main/delta/scan
LC = LW + K         # LayerNorm/mm1 domain (conv taps + even pad)
NT_D = D // 128     # 16 d-tiles
NT_K = D_OUTER // 128  # 8 k-tiles over d_outer
last_result = None
NG = 2              # n-groups for a/w/u/scan (pipelining granularity)
NH = N // NG        # n-values per group
# d-tiles whose hci/tree/gate run on POOL (engine balance; scan/u are
# DVE-only: the Pool engine does not implement TensorScalarPtr)
HC_DVE = {0, 5, 10, 15}


@functools.lru_cache(maxsize=2)
def _build_program(phases: str = "0ABCD"):
    import concourse.bass as bass
    import concourse.bacc as bacc
    import concourse.mybir as mybir
    import concourse.tile as tile
    from concourse.masks import make_identity

    f32 = mybir.dt.float32
    f16 = mybir.dt.float16
    AF = mybir.ActivationFunctionType
    OP = mybir.AluOpType

    # Steer the act-table-load pass: keep Exp and Ln only in their shared
    # set so phase C needs a single table load (ids/order preserved).
    import concourse.hw_specs as hw_specs
    if not getattr(bacc, "_act_tables_patched", False):
        _orig_gat = hw_specs.get_activation_tables

        def _gat(module_arch):
            tabs = _orig_gat(module_arch)
            AT = mybir.ActivationFunctionType
            for name, fns in tabs.items():
                if name != "natural_log_exp_and_others":
                    fns.discard(AT.Exp)
                    fns.discard(AT.Ln)
            return tabs

        bacc.get_activation_tables = _gat
        bacc._act_tables_patched = True

    nc = bacc.Bacc("TRN2", target_bir_lowering=False)

    # ---- DRAM I/O ----
    # Weights arrive pre-blocked so each per-d-tile stream is one contiguous
    # [128, contraction*128] read (2KB+ rows -> full DMA bandwidth).
    Xs_d = nc.dram_tensor("Xs", [LC, D_OUTER], f32, kind="ExternalInput")
    W1P_d = nc.dram_tensor("W1P", [NT_D * 128, NT_K * 128], f16,
                           kind="ExternalInput")
    W2P_d = nc.dram_tensor("W2P", [NT_D * 128, NT_K * 128], f16,
                           kind="ExternalInput")
    WllP_d = nc.dram_tensor("WllP", [NT_D * 128, NT_D * 128], f16,
                            kind="ExternalInput")
    WbcP_d = nc.dram_tensor("WbcP", [128, NT_D * 2 * N], f16,
                            kind="ExternalInput")
    WdP_d = nc.dram_tensor("WdP", [NT_K * 128, NT_D * 128], f16,
                           kind="ExternalInput")
    # packed per-d-tile constants: [128, NT_D, K+3+N]
    # (conv taps, conv bias, delta bias, gate bias, A row)
    CW = K + 3 + N
    cpk_d = nc.dram_tensor("cpk", [128, NT_D * CW], f32, kind="ExternalInput")
    # packed [2N, 2]: col 0 = b_ll[D:], col 1 = 1/A (B rows) or 1 (C rows)
    bvk_d = nc.dram_tensor("bvk", [2 * N, 2], f32, kind="ExternalInput")
    # packed [128, NT_K]: col e8 = b_down[e8*128:(e8+1)*128]
    bdown_d = nc.dram_tensor("bdown", [128, NT_K], f32, kind="ExternalInput")
    mask_d = nc.dram_tensor("mask", [1, LW], f32, kind="ExternalInput")
    Y_d = nc.dram_tensor("Y", [D_OUTER, LO], f32, kind="ExternalOutput")

    def bcast_n(t, nrep):
        # stride-0 broadcast of a [128, F] tile to [128, nrep, F]
        return bass.AP(tensor=t.tensor, offset=t.offset,
                       ap=[t.ap[0], [0, nrep], t.ap[1]])

    with tile.TileContext(nc) as tc:
        with (
            tc.tile_pool(name="const", bufs=1) as const,
            tc.tile_pool(name="persist", bufs=1) as persist,
            tc.tile_pool(name="work", bufs=2) as work,
            tc.tile_pool(name="big", bufs=2) as big,
            tc.tile_pool(name="bigwu", bufs=2) as bigwu,
            tc.tile_pool(name="red", bufs=2) as red,
            tc.tile_pool(name="wstream", bufs=2) as wstream,
            tc.tile_pool(name="psT", bufs=2, space="PSUM") as psT,
            tc.tile_pool(name="psB", bufs=1, space="PSUM") as psB,
            tc.tile_pool(name="psD", bufs=1, space="PSUM") as psD,
            tc.tile_pool(name="psA", bufs=4, space="PSUM") as psA,
        ):
            # ---- Phase 0 pool (row tiles processed one at a time) ----
            rows = [128, 128, LC - 256]
            p0_cm = tc.tile_pool(name="p0", bufs=2)
            p0 = p0_cm.__enter__()

            # ---- constants (packed DMAs) ----
            ident = const.tile([128, 128], f16, tag="ident")
            make_identity(nc, ident)
            eps_sb = const.tile([128, 1], f32, tag="eps")
            nc.vector.memset(eps_sb, 1e-5)

            cpk_sb = const.tile([128, NT_D, CW], f32, tag="cpk")
            nc.sync.dma_start(
                out=cpk_sb, in_=cpk_d.rearrange("p (dt f) -> p dt f", f=CW))
            convw_sb = [cpk_sb[:, dt, 0:K] for dt in range(NT_D)]
            cb2_sb = [cpk_sb[:, dt, K:K + 1] for dt in range(NT_D)]
            bd_sb = [cpk_sb[:, dt, K + 1:K + 2] for dt in range(NT_D)]
            c2_sb = [cpk_sb[:, dt, K + 2:K + 3] for dt in range(NT_D)]
            A_sb = [cpk_sb[:, dt, K + 3:K + 3 + N] for dt in range(NT_D)]

            bvk_sb = const.tile([2 * N, 2], f32, tag="bvk")
            nc.sync.dma_start(out=bvk_sb, in_=bvk_d[:, :])
            bbc_sb = bvk_sb[:, 0:1]
            invAv_sb = bvk_sb[:, 1:2]
            mask_sb = const.tile([2 * N, LW], f32, tag="mask")
            m_ap = mask_d[:, :]
            nc.sync.dma_start(
                out=mask_sb,
                in_=bass.AP(tensor=m_ap.tensor, offset=m_ap.offset,
                            ap=[[0, 2 * N], m_ap.ap[1]]))
            bdown_pk = const.tile([128, NT_K], f32, tag="bdn")
            nc.sync.dma_start(out=bdown_pk, in_=bdown_d[:, :])
            bdown_sb = [bdown_pk[:, e8:e8 + 1] for e8 in range(NT_K)]

            # ---- Phase 0: per row-tile: load, LayerNorm, transpose ----
            xhatT = []
            for kt in range(NT_K):
                xt = persist.tile([128, LC], f16, tag=f"xhT{kt}")
                xhatT.append(xt)
            dres_cm = tc.tile_pool(name="dres", bufs=1, space="DRAM")
            drp = dres_cm.__enter__()
            mu_d = drp.tile([3 * 128, 1], f32, tag="mu_d")
            sig_d = drp.tile([3 * 128, 1], f32, tag="sig_d")
            for i in range(3):
                r = rows[i]
                xr = p0.tile([128, D_OUTER], f32, tag="xr")
                # 4 chunked DMAs per row-tile to spread across DMA engines
                step = (r + 3) // 4
                for c0 in range(0, r, step):
                    c1 = min(c0 + step, r)
                    nc.sync.dma_start(
                        out=xr[c0:c1, :],
                        in_=Xs_d[i * 128 + c0:i * 128 + c1, :])
                # bn_stats free-dim max is 512: two subgroups then aggregate
                stats = work.tile([128, 2, 6], f32, tag="stats")
                for sg in range(2):
                    nc.vector.bn_stats(out=stats[:r, sg, :],
                                       in_=xr[:r, sg * 512:(sg + 1) * 512])
                mv = work.tile([128, 2], f32, tag="mv")
                nc.vector.bn_aggr(out=mv[:r, :], in_=stats[:r, :, :])
                sig = work.tile([128, 1], f32, tag="sig")
                nc.scalar.activation(out=sig[:r], in_=mv[:r, 1:2],
                                     func=AF.Sqrt, bias=eps_sb[:r, 0:1],
                                     scale=1.0)
                rsig = work.tile([128, 1], f32, tag="rsig")
                nc.vector.reciprocal(out=rsig[:r], in_=sig[:r])
                nmu = work.tile([128, 1], f32, tag="nmu")
                nc.vector.tensor_scalar(out=nmu[:r], in0=mv[:r, 0:1],
                                        scalar1=rsig[:r, 0:1], scalar2=-1.0,
                                        op0=OP.mult, op1=OP.mult)
                xh = p0.tile([128, D_OUTER], f16, tag="xh")
                nc.vector.tensor_scalar(out=xh[:r, :], in0=xr[:r, :],
                                        scalar1=rsig[:r, 0:1],
                                        scalar2=nmu[:r, 0:1],
                                        op0=OP.mult, op1=OP.add)
                # stage mu/sig to DRAM (read back broadcast for the residual)
                nc.sync.dma_start(out=mu_d[i * 128:i * 128 + r, :],
                                  in_=mv[:r, 0:1])
                nc.sync.dma_start(out=sig_d[i * 128:i * 128 + r, :],
                                  in_=sig[:r])
                for kt in range(NT_K):
                    cs = slice(kt * 128, (kt + 1) * 128)
                    pt = psT.tile([128, 128], f16, tag="tp")
                    nc.tensor.transpose(pt[:, :r], xh[:r, cs],
                                        ident[:r, :r])
                    nc.scalar.copy(out=xhatT[kt][:, i * 128:i * 128 + r],
                                   in_=pt[:, :r])
            mu_bc = persist.tile([128, LO], f32, tag="mu_bc")
            sig_bc = persist.tile([128, LO], f32, tag="sig_bc")
            own0 = WARM + K - 1
            for (dst, srcd) in ((mu_bc, mu_d), (sig_bc, sig_d)):
                s_ap = srcd[own0:own0 + LO, :]
                nc.sync.dma_start(
                    out=dst,
                    in_=bass.AP(tensor=s_ap.tensor, offset=s_ap.offset,
                                ap=[[0, 128], [1, LO]]))
            dres_cm.__exit__(None, None, None)
            p0_cm.__exit__(None, None, None)

            # ---- Phase A (+A2+B interleaved per d-tile) ----
            # mm1 + causal depthwise conv + silu -> X_main; gate mm2; and
            # the B/C projection accumulates incrementally so phase C can
            # start right after the last X_main tile.
            wbt = wstream.tile([128, NT_D, 2 * N], f16, tag="wbc")
            nc.sync.dma_start(
                out=wbt.rearrange("p kt e -> p (kt e)"),
                in_=WbcP_d[:, :])
            psbc = psB.tile([2 * N, LW], f32, tag="bc")
            X_main = []
            X_gate = []
            gate_silus = []
            for dt in range(NT_D if "A" in phases else 0):
                w1t = wstream.tile([128, NT_K, 128], f16, tag="w1")
                nc.sync.dma_start(
                    out=w1t.rearrange("p kt m -> p (kt m)"),
                    in_=W1P_d[dt * 128:(dt + 1) * 128, :])
                ps = psA.tile([128, LC], f32, tag="mm")
                for kt in range(NT_K):
                    nc.tensor.matmul(ps, w1t[:, kt, :],
                                     xhatT[kt],
                                     start=(kt == 0), stop=(kt == NT_K - 1))
                # depthwise conv: per-tap scaled copies on ACT (scale is the
                # per-channel tap weight), tap-sum via identity matmuls on PE
                pre16 = work.tile([128, K, LC], f16, tag="pre")
                for tap in range(K):
                    nc.scalar.activation(out=pre16[:, tap, :], in_=ps,
                                         func=AF.Identity,
                                         bias=0.0,
                                         scale=convw_sb[dt][:, tap:tap + 1])
                psC = psA.tile([128, LW], f32, tag="mm")
                for tap in range(K):
                    nc.tensor.matmul(psC, ident,
                                     pre16[:, tap, tap:tap + LW],
                                     start=(tap == 0), stop=(tap == K - 1))
                xm = persist.tile([128, LW], f16, tag=f"xm{dt}")
                nc.scalar.activation(out=xm, in_=psC, func=AF.Silu,
                                     bias=cb2_sb[dt][:, 0:1], scale=1.0)
                X_main.append(xm)
                # gate mm2 for this d-tile
                w2t = wstream.tile([128, NT_K, 128], f16, tag="w2")
                nc.sync.dma_start(
                    out=w2t.rearrange("p kt m -> p (kt m)"),
                    in_=W2P_d[dt * 128:(dt + 1) * 128, :])
                ps2 = psA.tile([128, LO], f32, tag="mm")
                for kt in range(NT_K):
                    nc.tensor.matmul(ps2, w2t[:, kt, :],
                                     xhatT[kt][:, WARM + K - 1:WARM + K - 1 + LO],
                                     start=(kt == 0), stop=(kt == NT_K - 1))
                xg = persist.tile([128, LO], f16, tag=f"xg{dt}")
                si = nc.scalar.activation(out=xg, in_=ps2, func=AF.Silu,
                                          bias=c2_sb[dt][:, 0:1], scale=1.0)
                gate_silus.append(si)
                X_gate.append(xg)
                # incremental B/C projection accumulate
                nc.tensor.matmul(psbc, wbt[:, dt, :], xm,
                                 start=(dt == 0), stop=(dt == NT_D - 1))

            # ---- Phase B: bias/scale/mask + partition-broadcast ----
            bc_raw = work.tile([2 * N, LW], f32, tag="bcraw")
            nc.scalar.activation(out=bc_raw, in_=psbc, func=AF.Identity,
                                 bias=bbc_sb[:, 0:1], scale=1.0)
            bci = work.tile([2 * N, LW], f16, tag="bci")
            nc.vector.scalar_tensor_tensor(out=bci, in0=bc_raw,
                                           scalar=invAv_sb[:, 0:1],
                                           in1=mask_sb, op0=OP.mult,
                                           op1=OP.mult)
            Bm_bcI = persist.tile([128, N, LW], f16, tag="BmbcI")
            Cm_bc = persist.tile([128, N, LO], f16, tag="Cmbc")
            with tc.tile_pool(name="dstage", bufs=1, space="DRAM") as dpool:
                bci_dram = dpool.tile([2 * N, LW], f16, tag="bcid")
                nc.sync.dma_start(out=bci_dram, in_=bci)
                for n in range(N):
                    src_b = bci_dram[n:n + 1, :]
                    nc.sync.dma_start(
                        out=Bm_bcI[:, n, :],
                        in_=bass.AP(tensor=src_b.tensor, offset=src_b.offset,
                                    ap=[[0, 128]] + src_b.ap[1:]))
                    src_c = bci_dram[N + n:N + n + 1, WARM:LW]
                    nc.sync.dma_start(
                        out=Cm_bc[:, n, :],
                        in_=bass.AP(tensor=src_c.tensor, offset=src_c.offset,
                                    ap=[[0, 128]] + src_c.ap[1:]))

            # residual precompute: xres = xhatT*sig + mu + b_down (per e8)
            xres = []
            for e8 in range(NT_K):
                xrec = work.tile([128, LO], f32, tag="xrec")
                nc.gpsimd.tensor_tensor(out=xrec,
                                        in0=xhatT[e8]
                                        [:, WARM + K - 1:WARM + K - 1 + LO],
                                        in1=sig_bc, op=OP.mult)
                xr2 = persist.tile([128, LO], f32, tag=f"xres{e8}")
                nc.vector.scalar_tensor_tensor(
                    out=xr2, in0=xrec, scalar=bdown_sb[e8][:, 0:1],
                    in1=mu_bc, op0=OP.add, op1=OP.add)
                xres.append(xr2)

            # ---- Phase C: per d-tile: delta, a, u, scan, y ----
            # (phase D partial accumulations interleave after dt 9 and 13)
            y_gated = []
            xy1 = []

            def emit_phase_d_part(d0, d1, first):
                # accumulate sum_dt WdT.yg for dt in [d0,d1) into xy1[e8]
                for e8 in range(NT_K):
                    wdt = wstream.tile([128, d1 - d0, 128], f16,
                                       tag="wd8" if d1 - d0 == 4 else "wd0")
                    nc.sync.dma_start(
                        out=wdt.rearrange("p dt m -> p (dt m)"),
                        in_=WdP_d[e8 * 128:(e8 + 1) * 128,
                                  d0 * 128:d1 * 128])
                    ps = psD.tile([128, LO], f32, tag="mm")
                    for i in range(d1 - d0):
                        nc.tensor.matmul(ps, wdt[:, i, :], y_gated[d0 + i],
                                         start=(i == 0), stop=(i == d1 - d0 - 1))
                    xy = work.tile([128, LO], f32, tag="xy")
                    nc.scalar.activation(out=xy, in_=ps, func=AF.Identity,
                                         bias=0.0, scale=1.0)
                    if first:
                        xy2 = persist.tile([128, LO], f32, tag=f"xy2_{e8}")
                        nc.gpsimd.tensor_tensor(out=xy2, in0=xy,
                                                in1=xres[e8], op=OP.add)
                        xy1.append(xy2)
                    else:
                        nc.gpsimd.tensor_tensor(out=xy1[e8], in0=xy,
                                                in1=xy1[e8], op=OP.add)

            for dt in range(NT_D):
                wllt = wstream.tile([128, NT_D, 128], f16, tag="wst")
                nc.sync.dma_start(
                    out=wllt.rearrange("p kt m -> p (kt m)"),
                    in_=WllP_d[dt * 128:(dt + 1) * 128, :])
                ps = psA.tile([128, LW], f32, tag="mm")
                for kt in range(NT_D):
                    nc.tensor.matmul(ps, wllt[:, kt, :],
                                     X_main[kt],
                                     start=(kt == 0), stop=(kt == NT_D - 1))
                # softplus(x) = ln(exp(x) + 1); exp & ln share one ACT table set
                e1 = work.tile([128, LW], f32, tag="e1")
                e1i = nc.scalar.activation(out=e1, in_=ps, func=AF.Exp,
                                           bias=bd_sb[dt][:, 0:1], scale=1.0)
                if dt == 0:
                    from concourse.tile_rust import add_dep_helper
                    for si in gate_silus:
                        add_dep_helper(e1i.ins, si.ins, False,
                                       "ACT table-set phase ordering")
                delta = work.tile([128, LW], f32, tag="delta")
                nc.scalar.activation(out=delta, in_=e1, func=AF.Ln,
                                     bias=1.0, scale=1.0)

                hc_eng = nc.vector if dt in HC_DVE else nc.gpsimd
                hci = red.tile([128, N, LO], f16, tag="hci")
                for g in range(NG):
                    ns = slice(g * NH, (g + 1) * NH)
                    a_t = big.tile([128, NH, LW], f16, tag=f"a{g}")
                    for i in range(NH):
                        n = g * NH + i
                        nc.scalar.activation(out=a_t[:, i, :], in_=delta,
                                             func=AF.Exp, bias=0.0,
                                             scale=A_sb[dt][:, n:n + 1])
                    w_t = bigwu.tile([128, NH, LW], f16, tag=f"w{g}")
                    nc.vector.tensor_tensor(
                        out=w_t, in0=bcast_n(X_main[dt], NH),
                        in1=Bm_bcI[:, ns, :], op=OP.mult)
                    # u = (a-1)*w as TS(4x) + in-place TT(2x): beats one STT
                    u_t = bigwu.tile([128, NH, LW], f16, tag=f"u{g}")
                    nc.vector.tensor_scalar(
                        out=u_t.rearrange("p n l -> p (n l)"),
                        in0=a_t.rearrange("p n l -> p (n l)"),
                        scalar1=-1.0, scalar2=None, op0=OP.add)
                    nc.vector.tensor_tensor(
                        out=u_t, in0=u_t, in1=w_t, op=OP.mult)
                    # zero decay at each n-segment start: encodes h(start)=u
                    nc.vector.memset(a_t[:, :, 0:1], 0.0)
                    # scan writes over w_t (dead once u is formed)
                    h_t = w_t
                    nc.vector.tensor_tensor_scan(
                        out=h_t.rearrange("p n l -> p (n l)"),
                        data0=a_t.rearrange("p n l -> p (n l)"),
                        data1=u_t.rearrange("p n l -> p (n l)"),
                        initial=0.0, op0=OP.mult, op1=OP.add)
                    hc_eng.tensor_tensor(
                        out=hci[:, ns, :],
                        in0=h_t[:, :, WARM:LW], in1=Cm_bc[:, ns, :],
                        op=OP.mult)
                # reduce over n: pairwise tree, in place in the low half
                # of hci (fp16 is 2x on DVE)
                for lv in (2, 4, 8):
                    hc_eng.tensor_tensor(out=hci[:, 0:N // lv, :],
                                         in0=hci[:, 0:N // lv, :],
                                         in1=hci[:, N // lv:2 * N // lv, :],
                                         op=OP.add)
                ysum = red.tile([128, LO], f16, tag="ysum")
                hc_eng.tensor_tensor(out=ysum, in0=hci[:, 0, :],
                                     in1=hci[:, 1, :], op=OP.add)
                yg = persist.tile([128, LO], f16, tag=f"yg{dt}")
                hc_eng.tensor_tensor(out=yg, in0=ysum, in1=X_gate[dt],
                                     op=OP.mult)
                y_gated.append(yg)
                if dt == 9:
                    emit_phase_d_part(0, 8, first=True)
                elif dt == 13:
                    emit_phase_d_part(8, 12, first=False)

            # ---- Phase D (last 4 d-tiles): down projection + residual ----
            for e8 in range(NT_K):
                wdt = wstream.tile([128, 4, 128], f16, tag="wd8")
                nc.sync.dma_start(
                    out=wdt.rearrange("p dt m -> p (dt m)"),
                    in_=WdP_d[e8 * 128:(e8 + 1) * 128, 12 * 128:])
                ps = psD.tile([128, LO], f32, tag="mm")
                for i in range(4):
                    nc.tensor.matmul(ps, wdt[:, i, :],
                                     y_gated[12 + i],
                                     start=(i == 0), stop=(i == 3))
                osb = work.tile([128, LO], f32, tag="osb")
                nc.vector.tensor_tensor(out=osb, in0=ps, in1=xy1[e8],
                                        op=OP.add)
                nc.sync.dma_start(out=Y_d[e8 * 128:(e8 + 1) * 128, :], in_=osb)

    nc.compile()
    return nc


def kernel(X, ln_g, ln_b, W_up1, conv_w, conv_b, W_ll, b_ll, A_log, W_up2,
           W_down, b_down):
    from concourse.bass_utils import run_bass_kernel_spmd

    f = np.float32
    h = np.float16
    X = np.asarray(X, f)
    A = -np.exp(np.asarray(A_log, f))
    assert np.allclose(A, A[0:1, :]), "kernel assumes A rows identical"
    c1 = (np.asarray(W_up1, f) @ np.asarray(ln_b, f)).astype(f)
    c2 = (np.asarray(W_up2, f) @ np.asarray(ln_b, f)).astype(f)
    cw = np.asarray(conv_w, f)[:, 0, :]                      # [D, K]
    cb2 = (np.asarray(conv_b, f) + c1 * cw.sum(1)).astype(f)
    # packed per-d-tile constants [128, NT_D, K+3+N]
    CW = K + 3 + N
    bd_ll = np.asarray(b_ll, f)
    cpk = np.empty((128, D // 128, CW), f)
    for dt in range(D // 128):
        r = slice(dt * 128, (dt + 1) * 128)
        cpk[:, dt, 0:K] = cw[r]
        cpk[:, dt, K] = cb2[r]
        cpk[:, dt, K + 1] = bd_ll[:D][r]
        cpk[:, dt, K + 2] = c2[r]
        cpk[:, dt, K + 3:] = A[r]
    bvk = np.stack([bd_ll[D:],
                    np.concatenate([1.0 / A[0], np.ones(N, f)])], axis=1)
    def block(w, nt_out):
        # [C, E] -> [E//128*128, C] blocked: out[e_t*128+p, c_t*128+m]
        # = w[c_t*128+p, e_t*128+m]  (per-e-tile contiguous weight stream)
        C, E = w.shape
        return np.ascontiguousarray(
            w.reshape(C // 128, 128, E // 128, 128)
             .transpose(2, 1, 0, 3).reshape(E, C))

    W1T = (np.asarray(W_up1, f) * np.asarray(ln_g, f)[None, :]).T.astype(h)
    W2T = (np.asarray(W_up2, f) * np.asarray(ln_g, f)[None, :]).T.astype(h)
    WllT = np.asarray(W_ll, f).T.astype(h)
    WdT = np.asarray(W_down, f).T.astype(h)
    shared = {
        "W1P": block(W1T, D // 128),
        "W2P": block(W2T, D // 128),
        "WllP": block(WllT[:, :D], D // 128),
        "WbcP": np.ascontiguousarray(
            WllT[:, D:].reshape(D // 128, 128, 2 * N)
            .transpose(1, 0, 2).reshape(128, -1)),
        "WdP": block(WdT, D_OUTER // 128),
        "cpk": np.ascontiguousarray(cpk.reshape(128, -1)),
        "bvk": np.ascontiguousarray(bvk.astype(f)),
        "bdown": np.ascontiguousarray(
            np.asarray(b_down, f).reshape(D_OUTER // 128, 128).T),
    }
    in_maps = []
    for c in range(NCORES):
        b, q = divmod(c, 4)
        l0 = q * LO
        lo_ext = l0 - (WARM + K - 1)
        xs = np.zeros((LC, D_OUTER), f)
        src0 = max(0, lo_ext)
        hi = min(l0 + LO + 1, L)
        xs[src0 - lo_ext:src0 - lo_ext + (hi - src0), :] = X[b, src0:hi, :]
        mask = np.ones((1, LW), f)
        if q == 0:
            mask[0, :WARM] = 0.0
        in_maps.append({"Xs": xs, "mask": mask, **shared})

    nc = _build_program()
    res = run_bass_kernel_spmd(nc, in_maps, core_ids=list(range(NCORES)))
    global last_result
    last_result = res

    out = np.empty((B_SZ, L, D_OUTER), f)
    for c in range(NCORES):
        b, q = divmod(c, 4)
        out[b, q * LO:(q + 1) * LO, :] = res.results[c]["Y"].T
    return out


# revision 62
# speedup vs baseline: 1.1922x; 1.0326x over previous
"""Trainium2 Bass kernel for a Mamba-1-style MixerBlock.

Reference computation (shapes: X[2,1024,1024], D=2048, N=16, K=4):
  Xn = LayerNorm(X) * g + b
  X_main = silu(conv_b + causal_depthwise_conv1d(Xn @ W_up1.T))
  pp = X_main @ W_ll.T + b_ll ; delta = softplus(pp[:, :D]); Bm, Cm = pp[:, D:D+N], pp[:, D+N:]
  a = exp(delta * A)  (A = -exp(A_log), [D,N])
  u = (a-1)/A * Bm * X_main        (per (b,l,d,n))
  h[t] = a[t] h[t-1] + u[t]        (scan over L per (b,d,n))
  y_ssm[t,d] = sum_n Cm[t,n] h[t,d,n]
  out = X + (y_ssm * silu(Xn @ W_up2.T)) @ W_down.T + b_down

Sharding: sequence-parallel over 8 cores (2 batches x 4 L-quarters of 256).
Each core redundantly recomputes a WARM-step scan warmup (decays are fast),
so the kernel is embarrassingly parallel - no collectives.

Per-core layout: channels on partitions, sequence on the free dim.
All matmuls run in fp16 (PE 1 cycle/row; fp32 PSUM accumulate); the SSM
elementwise chain runs in fp16 (DVE 2x for tensor_tensor; scan keeps an
fp32 internal state). The L-scan is a native tensor_tensor_scan chaining
n-segments per instruction (decay zeroed at segment starts encodes h=u).
Engine balance: scan on POOL, u split DVE/POOL, w/hci/tree/gate on DVE.
"""

import functools
import numpy as np

D_OUTER, D, N, K = 1024, 2048, 16, 4
B_SZ, L = 2, 1024
NCORES = 8
LO = 256            # own sequence steps per core
WARM = 24           # redundant scan warmup steps
LW = WARM + LO      # domain of X_main/delta/scan
LC = LW + K         # LayerNorm/mm1 domain (conv taps + even pad)
NT_D = D // 128     # 16 d-tiles
NT_K = D_OUTER // 128  # 8 k-tiles over d_outer
last_result = None
NG = 2              # n-groups for a/w/u/scan (pipelining granularity)
NH = N // NG        # n-values per group
# d-tiles whose hci/tree/gate run on POOL (engine balance; scan/u are
# DVE-only: the Pool engine does not implement TensorScalarPtr)
HC_DVE = {0, 3, 6, 9, 12}


@functools.lru_cache(maxsize=2)
def _build_program(phases: str = "0ABCD"):
    import concourse.bass as bass
    import concourse.bacc as bacc
    import concourse.mybir as mybir
    import concourse.tile as tile
    from concourse.masks import make_identity

    f32 = mybir.dt.float32
    f16 = mybir.dt.float16
    AF = mybir.ActivationFunctionType
    OP = mybir.AluOpType

    # Steer the act-table-load pass: keep Exp and Ln only in their shared
    # set so phase C needs a single table load (ids/order preserved).
    import concourse.hw_specs as hw_specs
    if not getattr(bacc, "_act_tables_patched", False):
        _orig_gat = hw_specs.get_activation_tables

        def _gat(module_arch):
            tabs = _orig_gat(module_arch)
            AT = mybir.ActivationFunctionType
            for name, fns in tabs.items():
                if name != "natural_log_exp_and_others":
                    fns.discard(AT.Exp)
                    fns.discard(AT.Ln)
            return tabs

        bacc.get_activation_tables = _gat
        bacc._act_tables_patched = True

    nc = bacc.Bacc("TRN2", target_bir_lowering=False)

    # ---- DRAM I/O ----
    # Weights arrive pre-blocked so each per-d-tile stream is one contiguous
    # [128, contraction*128] read (2KB+ rows -> full DMA bandwidth).
    Xs_d = nc.dram_tensor("Xs", [LC, D_OUTER], f32, kind="ExternalInput")
    W1P_d = nc.dram_tensor("W1P", [NT_D * 128, NT_K * 128], f16,
                           kind="ExternalInput")
    W2P_d = nc.dram_tensor("W2P", [NT_D * 128, NT_K * 128], f16,
                           kind="ExternalInput")
    WllP_d = nc.dram_tensor("WllP", [NT_D * 128, NT_D * 128], f16,
                            kind="ExternalInput")
    WbcP_d = nc.dram_tensor("WbcP", [128, NT_D * 2 * N], f16,
                            kind="ExternalInput")
    WdP_d = nc.dram_tensor("WdP", [NT_K * 128, NT_D * 128], f16,
                           kind="ExternalInput")
    # packed per-d-tile constants: [128, NT_D, K+4+N]
    # (conv taps, conv bias, delta bias, gate bias, -gate bias, A row)
    CW = K + 4 + N
    cpk_d = nc.dram_tensor("cpk", [128, NT_D * CW], f32, kind="ExternalInput")
    # packed [2N, 2]: col 0 = b_ll[D:], col 1 = 1/A (B rows) or 1 (C rows)
    bvk_d = nc.dram_tensor("bvk", [2 * N, 2], f32, kind="ExternalInput")
    # packed [128, NT_K]: col e8 = b_down[e8*128:(e8+1)*128]
    bdown_d = nc.dram_tensor("bdown", [128, NT_K], f32, kind="ExternalInput")
    mask_d = nc.dram_tensor("mask", [1, LW], f32, kind="ExternalInput")
    Y_d = nc.dram_tensor("Y", [D_OUTER, LO], f32, kind="ExternalOutput")

    def bcast_n(t, nrep):
        # stride-0 broadcast of a [128, F] tile to [128, nrep, F]
        return bass.AP(tensor=t.tensor, offset=t.offset,
                       ap=[t.ap[0], [0, nrep], t.ap[1]])

    with tile.TileContext(nc) as tc:
        with (
            tc.tile_pool(name="const", bufs=1) as const,
            tc.tile_pool(name="persist", bufs=1) as persist,
            tc.tile_pool(name="work", bufs=2) as work,
            tc.tile_pool(name="big", bufs=2) as big,
            tc.tile_pool(name="bigwu", bufs=2) as bigwu,
            tc.tile_pool(name="red", bufs=2) as red,
            tc.tile_pool(name="wstream", bufs=2) as wstream,
            tc.tile_pool(name="psT", bufs=2, space="PSUM") as psT,
            tc.tile_pool(name="psB", bufs=1, space="PSUM") as psB,
            tc.tile_pool(name="psD", bufs=1, space="PSUM") as psD,
            tc.tile_pool(name="psA", bufs=4, space="PSUM") as psA,
        ):
            # ---- Phase 0 pool (row tiles processed one at a time) ----
            rows = [128, 128, LC - 256]
            p0_cm = tc.tile_pool(name="p0", bufs=2)
            p0 = p0_cm.__enter__()

            # ---- constants (packed DMAs) ----
            ident = const.tile([128, 128], f16, tag="ident")
            make_identity(nc, ident)
            eps_sb = const.tile([128, 1], f32, tag="eps")
            nc.vector.memset(eps_sb, 1e-5)

            cpk_sb = const.tile([128, NT_D, CW], f32, tag="cpk")
            nc.sync.dma_start(
                out=cpk_sb, in_=cpk_d.rearrange("p (dt f) -> p dt f", f=CW))
            convw_sb = [cpk_sb[:, dt, 0:K] for dt in range(NT_D)]
            cb2_sb = [cpk_sb[:, dt, K:K + 1] for dt in range(NT_D)]
            bd_sb = [cpk_sb[:, dt, K + 1:K + 2] for dt in range(NT_D)]
            c2_sb = [cpk_sb[:, dt, K + 2:K + 3] for dt in range(NT_D)]
            nc2_sb = [cpk_sb[:, dt, K + 3:K + 4] for dt in range(NT_D)]
            A_sb = [cpk_sb[:, dt, K + 4:K + 4 + N] for dt in range(NT_D)]

            bvk_sb = const.tile([2 * N, 2], f32, tag="bvk")
            nc.sync.dma_start(out=bvk_sb, in_=bvk_d[:, :])
            bbc_sb = bvk_sb[:, 0:1]
            invAv_sb = bvk_sb[:, 1:2]
            mask_sb = const.tile([2 * N, LW], f32, tag="mask")
            m_ap = mask_d[:, :]
            nc.sync.dma_start(
                out=mask_sb,
                in_=bass.AP(tensor=m_ap.tensor, offset=m_ap.offset,
                            ap=[[0, 2 * N], m_ap.ap[1]]))
            bdown_pk = const.tile([128, NT_K], f32, tag="bdn")
            nc.sync.dma_start(out=bdown_pk, in_=bdown_d[:, :])
            bdown_sb = [bdown_pk[:, e8:e8 + 1] for e8 in range(NT_K)]

            # ---- Phase 0: per row-tile: load, LayerNorm, transpose ----
            xhatT = []
            for kt in range(NT_K):
                xt = persist.tile([128, LC], f16, tag=f"xhT{kt}")
                xhatT.append(xt)
            dres_cm = tc.tile_pool(name="dres", bufs=1, space="DRAM")
            drp = dres_cm.__enter__()
            mu_d = drp.tile([3 * 128, 1], f32, tag="mu_d")
            sig_d = drp.tile([3 * 128, 1], f32, tag="sig_d")
            for i in range(3):
                r = rows[i]
                xr = p0.tile([128, D_OUTER], f32, tag="xr")
                # 4 chunked DMAs per row-tile to spread across DMA engines
                step = (r + 3) // 4
                for c0 in range(0, r, step):
                    c1 = min(c0 + step, r)
                    nc.sync.dma_start(
                        out=xr[c0:c1, :],
                        in_=Xs_d[i * 128 + c0:i * 128 + c1, :])
                # bn_stats free-dim max is 512: two subgroups then aggregate
                stats = work.tile([128, 2, 6], f32, tag="stats")
                for sg in range(2):
                    nc.vector.bn_stats(out=stats[:r, sg, :],
                                       in_=xr[:r, sg * 512:(sg + 1) * 512])
                mv = work.tile([128, 2], f32, tag="mv")
                nc.vector.bn_aggr(out=mv[:r, :], in_=stats[:r, :, :])
                sig = work.tile([128, 1], f32, tag="sig")
                nc.scalar.activation(out=sig[:r], in_=mv[:r, 1:2],
                                     func=AF.Sqrt, bias=eps_sb[:r, 0:1],
                                     scale=1.0)
                rsig = work.tile([128, 1], f32, tag="rsig")
                nc.vector.reciprocal(out=rsig[:r], in_=sig[:r])
                nmu = work.tile([128, 1], f32, tag="nmu")
                nc.vector.tensor_scalar(out=nmu[:r], in0=mv[:r, 0:1],
                                        scalar1=rsig[:r, 0:1], scalar2=-1.0,
                                        op0=OP.mult, op1=OP.mult)
                xh = p0.tile([128, D_OUTER], f16, tag="xh")
                nc.vector.tensor_scalar(out=xh[:r, :], in0=xr[:r, :],
                                        scalar1=rsig[:r, 0:1],
                                        scalar2=nmu[:r, 0:1],
                                        op0=OP.mult, op1=OP.add)
                # stage mu/sig to DRAM (read back broadcast for the residual)
                nc.sync.dma_start(out=mu_d[i * 128:i * 128 + r, :],
                                  in_=mv[:r, 0:1])
                nc.sync.dma_start(out=sig_d[i * 128:i * 128 + r, :],
                                  in_=sig[:r])
                for kt in range(NT_K):
                    cs = slice(kt * 128, (kt + 1) * 128)
                    pt = psT.tile([128, 128], f16, tag="tp")
                    nc.tensor.transpose(pt[:, :r], xh[:r, cs],
                                        ident[:r, :r])
                    nc.scalar.copy(out=xhatT[kt][:, i * 128:i * 128 + r],
                                   in_=pt[:, :r])
            mu_bc = persist.tile([128, LO], f32, tag="mu_bc")
            sig_bc = persist.tile([128, LO], f32, tag="sig_bc")
            own0 = WARM + K - 1
            for (dst, srcd) in ((mu_bc, mu_d), (sig_bc, sig_d)):
                s_ap = srcd[own0:own0 + LO, :]
                nc.sync.dma_start(
                    out=dst,
                    in_=bass.AP(tensor=s_ap.tensor, offset=s_ap.offset,
                                ap=[[0, 128], [1, LO]]))
            dres_cm.__exit__(None, None, None)
            p0_cm.__exit__(None, None, None)

            # ---- Phase A (+A2+B interleaved per d-tile) ----
            # mm1 + causal depthwise conv + silu -> X_main; gate mm2; and
            # the B/C projection accumulates incrementally so phase C can
            # start right after the last X_main tile.
            wbt = wstream.tile([128, NT_D, 2 * N], f16, tag="wbc")
            nc.sync.dma_start(
                out=wbt.rearrange("p kt e -> p (kt e)"),
                in_=WbcP_d[:, :])
            psbc = psB.tile([2 * N, LW], f32, tag="bc")
            X_main = []
            X_gate = []
            gate_silus = []
            for dt in range(NT_D if "A" in phases else 0):
                w1t = wstream.tile([128, NT_K, 128], f16, tag="w1")
                nc.sync.dma_start(
                    out=w1t.rearrange("p kt m -> p (kt m)"),
                    in_=W1P_d[dt * 128:(dt + 1) * 128, :])
                ps = psA.tile([128, LC], f32, tag="mm")
                for kt in range(NT_K):
                    nc.tensor.matmul(ps, w1t[:, kt, :],
                                     xhatT[kt],
                                     start=(kt == 0), stop=(kt == NT_K - 1))
                # depthwise causal conv: tap-accumulation STT chain on DVE
                # (DVE is otherwise idle during phase A)
                acc = None
                for tap in range(K):
                    nxt = work.tile([128, LW], f32, tag="cacc")
                    if acc is None:
                        nc.vector.tensor_scalar(
                            out=nxt, in0=ps[:, tap:tap + LW],
                            scalar1=convw_sb[dt][:, tap:tap + 1], scalar2=None,
                            op0=OP.mult)
                    else:
                        nc.vector.scalar_tensor_tensor(
                            out=nxt, in0=ps[:, tap:tap + LW],
                            scalar=convw_sb[dt][:, tap:tap + 1], in1=acc,
                            op0=OP.mult, op1=OP.add)
                    acc = nxt
                xm = persist.tile([128, LW], f16, tag=f"xm{dt}")
                si = nc.scalar.activation(out=xm, in_=acc, func=AF.Silu,
                                          bias=cb2_sb[dt][:, 0:1], scale=1.0)
                gate_silus.append(si)
                X_main.append(xm)
                # incremental B/C projection accumulate
                nc.tensor.matmul(psbc, wbt[:, dt, :], xm,
                                 start=(dt == 0), stop=(dt == NT_D - 1))

            # ---- Phase B: bias/scale/mask + partition-broadcast ----
            bc_raw = work.tile([2 * N, LW], f32, tag="bcraw")
            nc.scalar.activation(out=bc_raw, in_=psbc, func=AF.Identity,
                                 bias=bbc_sb[:, 0:1], scale=1.0)
            bci = work.tile([2 * N, LW], f16, tag="bci")
            nc.vector.scalar_tensor_tensor(out=bci, in0=bc_raw,
                                           scalar=invAv_sb[:, 0:1],
                                           in1=mask_sb, op0=OP.mult,
                                           op1=OP.mult)
            Bm_bcI = persist.tile([128, N, LW], f16, tag="BmbcI")
            Cm_bc = persist.tile([128, N, LO], f16, tag="Cmbc")
            with tc.tile_pool(name="dstage", bufs=1, space="DRAM") as dpool:
                bci_dram = dpool.tile([2 * N, LW], f16, tag="bcid")
                nc.sync.dma_start(out=bci_dram, in_=bci)
                for n in range(N):
                    src_b = bci_dram[n:n + 1, :]
                    nc.sync.dma_start(
                        out=Bm_bcI[:, n, :],
                        in_=bass.AP(tensor=src_b.tensor, offset=src_b.offset,
                                    ap=[[0, 128]] + src_b.ap[1:]))
                    src_c = bci_dram[N + n:N + n + 1, WARM:LW]
                    nc.sync.dma_start(
                        out=Cm_bc[:, n, :],
                        in_=bass.AP(tensor=src_c.tensor, offset=src_c.offset,
                                    ap=[[0, 128]] + src_c.ap[1:]))

            # residual precompute: xres = xhatT*sig + mu + b_down (per e8)
            xres = []
            for e8 in range(NT_K):
                xrec = work.tile([128, LO], f32, tag="xrec")
                nc.gpsimd.tensor_tensor(out=xrec,
                                        in0=xhatT[e8]
                                        [:, WARM + K - 1:WARM + K - 1 + LO],
                                        in1=sig_bc, op=OP.mult)
                xr2 = persist.tile([128, LO], f32, tag=f"xres{e8}")
                nc.vector.scalar_tensor_tensor(
                    out=xr2, in0=xrec, scalar=bdown_sb[e8][:, 0:1],
                    in1=mu_bc, op0=OP.add, op1=OP.add)
                xres.append(xr2)

            # ---- Phase C: per d-tile: delta, a, u, scan, y ----
            # (phase D partial accumulations interleave after dt 9 and 13)
            y_gated = []
            xy1 = []

            def emit_phase_d_part(d0, d1, first):
                # accumulate sum_dt WdT.yg for dt in [d0,d1) into xy1[e8]
                for e8 in range(NT_K):
                    wdt = wstream.tile([128, d1 - d0, 128], f16,
                                       tag="wd8" if d1 - d0 == 4 else "wd0")
                    nc.sync.dma_start(
                        out=wdt.rearrange("p dt m -> p (dt m)"),
                        in_=WdP_d[e8 * 128:(e8 + 1) * 128,
                                  d0 * 128:d1 * 128])
                    ps = psD.tile([128, LO], f32, tag="mm")
                    for i in range(d1 - d0):
                        nc.tensor.matmul(ps, wdt[:, i, :], y_gated[d0 + i],
                                         start=(i == 0), stop=(i == d1 - d0 - 1))
                    xy = work.tile([128, LO], f32, tag="xy")
                    nc.scalar.activation(out=xy, in_=ps, func=AF.Identity,
                                         bias=0.0, scale=1.0)
                    if first:
                        xy2 = persist.tile([128, LO], f32, tag=f"xy2_{e8}")
                        nc.gpsimd.tensor_tensor(out=xy2, in0=xy,
                                                in1=xres[e8], op=OP.add)
                        xy1.append(xy2)
                    else:
                        nc.gpsimd.tensor_tensor(out=xy1[e8], in0=xy,
                                                in1=xy1[e8], op=OP.add)

            for dt in range(NT_D):
                # gate mm2 + silu built from the exp table:
                # silu(x) = x / (1 + exp(-x))  (avoids an ACT table switch)
                w2t = wstream.tile([128, NT_K, 128], f16, tag="w2")
                nc.sync.dma_start(
                    out=w2t.rearrange("p kt m -> p (kt m)"),
                    in_=W2P_d[dt * 128:(dt + 1) * 128, :])
                ps2 = psA.tile([128, LO], f32, tag="mm")
                for kt in range(NT_K):
                    nc.tensor.matmul(ps2, w2t[:, kt, :],
                                     xhatT[kt][:, WARM + K - 1:WARM + K - 1 + LO],
                                     start=(kt == 0), stop=(kt == NT_K - 1))
                eg = work.tile([128, LO], f16, tag="eg")
                nc.scalar.activation(out=eg, in_=ps2, func=AF.Exp,
                                     bias=nc2_sb[dt][:, 0:1], scale=-1.0)
                rg = work.tile([128, LO], f16, tag="rg")
                nc.vector.tensor_scalar(out=rg, in0=eg, scalar1=1.0,
                                        scalar2=None, op0=OP.add)
                with nc.allow_low_precision("sigmoid denominator, fp16 ok"):
                    nc.vector.reciprocal(out=rg, in_=rg)
                xg = persist.tile([128, LO], f16, tag=f"xg{dt}")
                nc.vector.scalar_tensor_tensor(
                    out=xg, in0=ps2, scalar=c2_sb[dt][:, 0:1], in1=rg,
                    op0=OP.add, op1=OP.mult)
                X_gate.append(xg)

                wllt = wstream.tile([128, NT_D, 128], f16, tag="wst")
                nc.sync.dma_start(
                    out=wllt.rearrange("p kt m -> p (kt m)"),
                    in_=WllP_d[dt * 128:(dt + 1) * 128, :])
                ps = psA.tile([128, LW], f32, tag="mm")
                for kt in range(NT_D):
                    nc.tensor.matmul(ps, wllt[:, kt, :],
                                     X_main[kt],
                                     start=(kt == 0), stop=(kt == NT_D - 1))
                # softplus(x) = ln(exp(x) + 1); exp & ln share one ACT table set
                e1 = work.tile([128, LW], f32, tag="e1")
                e1i = nc.scalar.activation(out=e1, in_=ps, func=AF.Exp,
                                           bias=bd_sb[dt][:, 0:1], scale=1.0)
                if dt == 0:
                    from concourse.tile_rust import add_dep_helper
                    for si in gate_silus:
                        add_dep_helper(e1i.ins, si.ins, False,
                                       "ACT table-set phase ordering")
                delta = work.tile([128, LW], f32, tag="delta")
                nc.scalar.activation(out=delta, in_=e1, func=AF.Ln,
                                     bias=1.0, scale=1.0)

                hc_eng = nc.vector if dt in HC_DVE else nc.gpsimd
                hci = red.tile([128, N, LO], f16, tag="hci")
                for g in range(NG):
                    ns = slice(g * NH, (g + 1) * NH)
                    a_t = big.tile([128, NH, LW], f16, tag=f"a{g}")
                    for i in range(NH):
                        n = g * NH + i
                        nc.scalar.activation(out=a_t[:, i, :], in_=delta,
                                             func=AF.Exp, bias=0.0,
                                             scale=A_sb[dt][:, n:n + 1])
                    w_t = bigwu.tile([128, NH, LW], f16, tag=f"w{g}")
                    nc.vector.tensor_tensor(
                        out=w_t, in0=bcast_n(X_main[dt], NH),
                        in1=Bm_bcI[:, ns, :], op=OP.mult)
                    # u = (a-1)*w as TS(4x) + in-place TT(2x): beats one STT
                    u_t = bigwu.tile([128, NH, LW], f16, tag=f"u{g}")
                    nc.vector.tensor_scalar(
                        out=u_t.rearrange("p n l -> p (n l)"),
                        in0=a_t.rearrange("p n l -> p (n l)"),
                        scalar1=-1.0, scalar2=None, op0=OP.add)
                    nc.vector.tensor_tensor(
                        out=u_t, in0=u_t, in1=w_t, op=OP.mult)
                    # zero decay at each n-segment start: encodes h(start)=u
                    nc.vector.memset(a_t[:, :, 0:1], 0.0)
                    # scan writes over w_t (dead once u is formed)
                    h_t = w_t
                    nc.vector.tensor_tensor_scan(
                        out=h_t.rearrange("p n l -> p (n l)"),
                        data0=a_t.rearrange("p n l -> p (n l)"),
                        data1=u_t.rearrange("p n l -> p (n l)"),
                        initial=0.0, op0=OP.mult, op1=OP.add)
                    hc_eng.tensor_tensor(
                        out=hci[:, ns, :],
                        in0=h_t[:, :, WARM:LW], in1=Cm_bc[:, ns, :],
                        op=OP.mult)
                # reduce over n: pairwise tree, in place in the low half
                # of hci (fp16 is 2x on DVE)
                for lv in (2, 4, 8):
                    hc_eng.tensor_tensor(out=hci[:, 0:N // lv, :],
                                         in0=hci[:, 0:N // lv, :],
                                         in1=hci[:, N // lv:2 * N // lv, :],
                                         op=OP.add)
                ysum = red.tile([128, LO], f16, tag="ysum")
                hc_eng.tensor_tensor(out=ysum, in0=hci[:, 0, :],
                                     in1=hci[:, 1, :], op=OP.add)
                yg = persist.tile([128, LO], f16, tag=f"yg{dt}")
                hc_eng.tensor_tensor(out=yg, in0=ysum, in1=X_gate[dt],
                                     op=OP.mult)
                y_gated.append(yg)
                if dt == 9:
                    emit_phase_d_part(0, 8, first=True)
                elif dt == 13:
                    emit_phase_d_part(8, 12, first=False)

            # ---- Phase D (last 4 d-tiles): down projection + residual ----
            for e8 in range(NT_K):
                wdt = wstream.tile([128, 4, 128], f16, tag="wd8")
                nc.sync.dma_start(
                    out=wdt.rearrange("p dt m -> p (dt m)"),
                    in_=WdP_d[e8 * 128:(e8 + 1) * 128, 12 * 128:])
                ps = psD.tile([128, LO], f32, tag="mm")
                for i in range(4):
                    nc.tensor.matmul(ps, wdt[:, i, :],
                                     y_gated[12 + i],
                                     start=(i == 0), stop=(i == 3))
                osb = work.tile([128, LO], f32, tag="osb")
                nc.vector.tensor_tensor(out=osb, in0=ps, in1=xy1[e8],
                                        op=OP.add)
                nc.sync.dma_start(out=Y_d[e8 * 128:(e8 + 1) * 128, :], in_=osb)

    nc.compile()
    return nc


def kernel(X, ln_g, ln_b, W_up1, conv_w, conv_b, W_ll, b_ll, A_log, W_up2,
           W_down, b_down):
    from concourse.bass_utils import run_bass_kernel_spmd

    f = np.float32
    h = np.float16
    X = np.asarray(X, f)
    A = -np.exp(np.asarray(A_log, f))
    assert np.allclose(A, A[0:1, :]), "kernel assumes A rows identical"
    c1 = (np.asarray(W_up1, f) @ np.asarray(ln_b, f)).astype(f)
    c2 = (np.asarray(W_up2, f) @ np.asarray(ln_b, f)).astype(f)
    cw = np.asarray(conv_w, f)[:, 0, :]                      # [D, K]
    cb2 = (np.asarray(conv_b, f) + c1 * cw.sum(1)).astype(f)
    # packed per-d-tile constants [128, NT_D, K+4+N]
    CW = K + 4 + N
    bd_ll = np.asarray(b_ll, f)
    cpk = np.empty((128, D // 128, CW), f)
    for dt in range(D // 128):
        r = slice(dt * 128, (dt + 1) * 128)
        cpk[:, dt, 0:K] = cw[r]
        cpk[:, dt, K] = cb2[r]
        cpk[:, dt, K + 1] = bd_ll[:D][r]
        cpk[:, dt, K + 2] = c2[r]
        cpk[:, dt, K + 3] = -c2[r]
        cpk[:, dt, K + 4:] = A[r]
    bvk = np.stack([bd_ll[D:],
                    np.concatenate([1.0 / A[0], np.ones(N, f)])], axis=1)
    def block(w, nt_out):
        # [C, E] -> [E//128*128, C] blocked: out[e_t*128+p, c_t*128+m]
        # = w[c_t*128+p, e_t*128+m]  (per-e-tile contiguous weight stream)
        C, E = w.shape
        return np.ascontiguousarray(
            w.reshape(C // 128, 128, E // 128, 128)
             .transpose(2, 1, 0, 3).reshape(E, C))

    W1T = (np.asarray(W_up1, f) * np.asarray(ln_g, f)[None, :]).T.astype(h)
    W2T = (np.asarray(W_up2, f) * np.asarray(ln_g, f)[None, :]).T.astype(h)
    WllT = np.asarray(W_ll, f).T.astype(h)
    WdT = np.asarray(W_down, f).T.astype(h)
    shared = {
        "W1P": block(W1T, D // 128),
        "W2P": block(W2T, D // 128),
        "WllP": block(WllT[:, :D], D // 128),
        "WbcP": np.ascontiguousarray(
            WllT[:, D:].reshape(D // 128, 128, 2 * N)
            .transpose(1, 0, 2).reshape(128, -1)),
        "WdP": block(WdT, D_OUTER // 128),
        "cpk": np.ascontiguousarray(cpk.reshape(128, -1)),
        "bvk": np.ascontiguousarray(bvk.astype(f)),
        "bdown": np.ascontiguousarray(
            np.asarray(b_down, f).reshape(D_OUTER // 128, 128).T),
    }
    in_maps = []
    for c in range(NCORES):
        b, q = divmod(c, 4)
        l0 = q * LO
        lo_ext = l0 - (WARM + K - 1)
        xs = np.zeros((LC, D_OUTER), f)
        src0 = max(0, lo_ext)
        hi = min(l0 + LO + 1, L)
        xs[src0 - lo_ext:src0 - lo_ext + (hi - src0), :] = X[b, src0:hi, :]
        mask = np.ones((1, LW), f)
        if q == 0:
            mask[0, :WARM] = 0.0
        in_maps.append({"Xs": xs, "mask": mask, **shared})

    nc = _build_program()
    res = run_bass_kernel_spmd(nc, in_maps, core_ids=list(range(NCORES)))
    global last_result
    last_result = res

    out = np.empty((B_SZ, L, D_OUTER), f)
    for c in range(NCORES):
        b, q = divmod(c, 4)
        out[b, q * LO:(q + 1) * LO, :] = res.results[c]["Y"].T
    return out


# revision 64
# speedup vs baseline: 1.2173x; 1.0210x over previous
"""Trainium2 Bass kernel for a Mamba-1-style MixerBlock.

Reference computation (shapes: X[2,1024,1024], D=2048, N=16, K=4):
  Xn = LayerNorm(X) * g + b
  X_main = silu(conv_b + causal_depthwise_conv1d(Xn @ W_up1.T))
  pp = X_main @ W_ll.T + b_ll ; delta = softplus(pp[:, :D]); Bm, Cm = pp[:, D:D+N], pp[:, D+N:]
  a = exp(delta * A)  (A = -exp(A_log), [D,N])
  u = (a-1)/A * Bm * X_main        (per (b,l,d,n))
  h[t] = a[t] h[t-1] + u[t]        (scan over L per (b,d,n))
  y_ssm[t,d] = sum_n Cm[t,n] h[t,d,n]
  out = X + (y_ssm * silu(Xn @ W_up2.T)) @ W_down.T + b_down

Sharding: sequence-parallel over 8 cores (2 batches x 4 L-quarters of 256).
Each core redundantly recomputes a WARM-step scan warmup (decays are fast),
so the kernel is embarrassingly parallel - no collectives.

Per-core layout: channels on partitions, sequence on the free dim.
All matmuls run in fp16 (PE 1 cycle/row; fp32 PSUM accumulate); the SSM
elementwise chain runs in fp16 (DVE 2x for tensor_tensor; scan keeps an
fp32 internal state). The L-scan is a native tensor_tensor_scan chaining
n-segments per instruction (decay zeroed at segment starts encodes h=u).
Engine balance: scan on POOL, u split DVE/POOL, w/hci/tree/gate on DVE.
"""

import functools
import numpy as np

D_OUTER, D, N, K = 1024, 2048, 16, 4
B_SZ, L = 2, 1024
NCORES = 8
LO = 256            # own sequence steps per core
WARM = 24           # redundant scan warmup steps
LW = WARM + LO      # domain of X_main/delta/scan
LC = LW + K         # LayerNorm/mm1 domain (conv taps + even pad)
NT_D = D // 128     # 16 d-tiles
NT_K = D_OUTER // 128  # 8 k-tiles over d_outer
last_result = None
NG = 2              # n-groups for a/w/u/scan (pipelining granularity)
NH = N // NG        # n-values per group
# d-tiles whose hci/tree/gate run on POOL (engine balance; scan/u are
# DVE-only: the Pool engine does not implement TensorScalarPtr)
HC_DVE = {0, 2, 4, 6, 8, 10, 13, 14, 15}


@functools.lru_cache(maxsize=2)
def _build_program(phases: str = "0ABCD"):
    import concourse.bass as bass
    import concourse.bacc as bacc
    import concourse.mybir as mybir
    import concourse.tile as tile
    from concourse.masks import make_identity

    f32 = mybir.dt.float32
    f16 = mybir.dt.float16
    AF = mybir.ActivationFunctionType
    OP = mybir.AluOpType

    # Steer the act-table-load pass: keep Exp and Ln only in their shared
    # set so phase C needs a single table load (ids/order preserved).
    import concourse.hw_specs as hw_specs
    if not getattr(bacc, "_act_tables_patched", False):
        _orig_gat = hw_specs.get_activation_tables

        def _gat(module_arch):
            tabs = _orig_gat(module_arch)
            AT = mybir.ActivationFunctionType
            for name, fns in tabs.items():
                if name != "natural_log_exp_and_others":
                    fns.discard(AT.Exp)
                    fns.discard(AT.Ln)
            return tabs

        bacc.get_activation_tables = _gat
        bacc._act_tables_patched = True

    nc = bacc.Bacc("TRN2", target_bir_lowering=False)

    # ---- DRAM I/O ----
    # Weights arrive pre-blocked so each per-d-tile stream is one contiguous
    # [128, contraction*128] read (2KB+ rows -> full DMA bandwidth).
    Xs_d = nc.dram_tensor("Xs", [LC, D_OUTER], f32, kind="ExternalInput")
    W1P_d = nc.dram_tensor("W1P", [NT_D * 128, NT_K * 128], f16,
                           kind="ExternalInput")
    W2P_d = nc.dram_tensor("W2P", [NT_D * 128, NT_K * 128], f16,
                           kind="ExternalInput")
    WllP_d = nc.dram_tensor("WllP", [NT_D * 128, NT_D * 128], f16,
                            kind="ExternalInput")
    WbcP_d = nc.dram_tensor("WbcP", [128, NT_D * 2 * N], f16,
                            kind="ExternalInput")
    WdP_d = nc.dram_tensor("WdP", [NT_K * 128, NT_D * 128], f16,
                           kind="ExternalInput")
    # packed per-d-tile constants: [128, NT_D, K+4+N]
    # (conv taps, conv bias, delta bias, gate bias, -gate bias, A row)
    CW = K + 4 + N
    cpk_d = nc.dram_tensor("cpk", [128, NT_D * CW], f32, kind="ExternalInput")
    # packed [2N, 2]: col 0 = b_ll[D:], col 1 = 1/A (B rows) or 1 (C rows)
    bvk_d = nc.dram_tensor("bvk", [2 * N, 2], f32, kind="ExternalInput")
    # packed [128, NT_K]: col e8 = b_down[e8*128:(e8+1)*128]
    bdown_d = nc.dram_tensor("bdown", [128, NT_K], f32, kind="ExternalInput")
    mask_d = nc.dram_tensor("mask", [1, LW], f32, kind="ExternalInput")
    Y_d = nc.dram_tensor("Y", [D_OUTER, LO], f32, kind="ExternalOutput")

    def bcast_n(t, nrep):
        # stride-0 broadcast of a [128, F] tile to [128, nrep, F]
        return bass.AP(tensor=t.tensor, offset=t.offset,
                       ap=[t.ap[0], [0, nrep], t.ap[1]])

    with tile.TileContext(nc) as tc:
        with (
            tc.tile_pool(name="const", bufs=1) as const,
            tc.tile_pool(name="persist", bufs=1) as persist,
            tc.tile_pool(name="work", bufs=2) as work,
            tc.tile_pool(name="big", bufs=2) as big,
            tc.tile_pool(name="bigwu", bufs=2) as bigwu,
            tc.tile_pool(name="red", bufs=2) as red,
            tc.tile_pool(name="wstream", bufs=2) as wstream,
            tc.tile_pool(name="psT", bufs=2, space="PSUM") as psT,
            tc.tile_pool(name="psB", bufs=1, space="PSUM") as psB,
            tc.tile_pool(name="psD", bufs=1, space="PSUM") as psD,
            tc.tile_pool(name="psA", bufs=4, space="PSUM") as psA,
        ):
            # ---- Phase 0 pool (row tiles processed one at a time) ----
            rows = [128, 128, LC - 256]
            p0_cm = tc.tile_pool(name="p0", bufs=2)
            p0 = p0_cm.__enter__()

            # ---- constants (packed DMAs) ----
            ident = const.tile([128, 128], f16, tag="ident")
            make_identity(nc, ident)
            eps_sb = const.tile([128, 1], f32, tag="eps")
            nc.vector.memset(eps_sb, 1e-5)

            cpk_sb = const.tile([128, NT_D, CW], f32, tag="cpk")
            nc.sync.dma_start(
                out=cpk_sb, in_=cpk_d.rearrange("p (dt f) -> p dt f", f=CW))
            convw_sb = [cpk_sb[:, dt, 0:K] for dt in range(NT_D)]
            cb2_sb = [cpk_sb[:, dt, K:K + 1] for dt in range(NT_D)]
            bd_sb = [cpk_sb[:, dt, K + 1:K + 2] for dt in range(NT_D)]
            c2_sb = [cpk_sb[:, dt, K + 2:K + 3] for dt in range(NT_D)]
            nc2_sb = [cpk_sb[:, dt, K + 3:K + 4] for dt in range(NT_D)]
            A_sb = [cpk_sb[:, dt, K + 4:K + 4 + N] for dt in range(NT_D)]

            bvk_sb = const.tile([2 * N, 2], f32, tag="bvk")
            nc.sync.dma_start(out=bvk_sb, in_=bvk_d[:, :])
            bbc_sb = bvk_sb[:, 0:1]
            invAv_sb = bvk_sb[:, 1:2]
            mask_sb = const.tile([2 * N, LW], f32, tag="mask")
            m_ap = mask_d[:, :]
            nc.sync.dma_start(
                out=mask_sb,
                in_=bass.AP(tensor=m_ap.tensor, offset=m_ap.offset,
                            ap=[[0, 2 * N], m_ap.ap[1]]))
            bdown_pk = const.tile([128, NT_K], f32, tag="bdn")
            nc.sync.dma_start(out=bdown_pk, in_=bdown_d[:, :])
            bdown_sb = [bdown_pk[:, e8:e8 + 1] for e8 in range(NT_K)]

            # ---- Phase 0: per row-tile: load, LayerNorm, transpose ----
            xhatT = []
            for kt in range(NT_K):
                xt = persist.tile([128, LC], f16, tag=f"xhT{kt}")
                xhatT.append(xt)
            dres_cm = tc.tile_pool(name="dres", bufs=1, space="DRAM")
            drp = dres_cm.__enter__()
            mu_d = drp.tile([3 * 128, 1], f32, tag="mu_d")
            sig_d = drp.tile([3 * 128, 1], f32, tag="sig_d")
            for i in range(3):
                r = rows[i]
                xr = p0.tile([128, D_OUTER], f32, tag="xr")
                # 4 chunked DMAs per row-tile to spread across DMA engines
                step = (r + 3) // 4
                for c0 in range(0, r, step):
                    c1 = min(c0 + step, r)
                    nc.sync.dma_start(
                        out=xr[c0:c1, :],
                        in_=Xs_d[i * 128 + c0:i * 128 + c1, :])
                # bn_stats free-dim max is 512: two subgroups then aggregate
                stats = work.tile([128, 2, 6], f32, tag="stats")
                for sg in range(2):
                    nc.vector.bn_stats(out=stats[:r, sg, :],
                                       in_=xr[:r, sg * 512:(sg + 1) * 512])
                mv = work.tile([128, 2], f32, tag="mv")
                nc.vector.bn_aggr(out=mv[:r, :], in_=stats[:r, :, :])
                sig = work.tile([128, 1], f32, tag="sig")
                nc.scalar.activation(out=sig[:r], in_=mv[:r, 1:2],
                                     func=AF.Sqrt, bias=eps_sb[:r, 0:1],
                                     scale=1.0)
                rsig = work.tile([128, 1], f32, tag="rsig")
                nc.vector.reciprocal(out=rsig[:r], in_=sig[:r])
                nmu = work.tile([128, 1], f32, tag="nmu")
                nc.vector.tensor_scalar(out=nmu[:r], in0=mv[:r, 0:1],
                                        scalar1=rsig[:r, 0:1], scalar2=-1.0,
                                        op0=OP.mult, op1=OP.mult)
                xh = p0.tile([128, D_OUTER], f16, tag="xh")
                nc.vector.tensor_scalar(out=xh[:r, :], in0=xr[:r, :],
                                        scalar1=rsig[:r, 0:1],
                                        scalar2=nmu[:r, 0:1],
                                        op0=OP.mult, op1=OP.add)
                # stage mu/sig to DRAM (read back broadcast for the residual)
                nc.sync.dma_start(out=mu_d[i * 128:i * 128 + r, :],
                                  in_=mv[:r, 0:1])
                nc.sync.dma_start(out=sig_d[i * 128:i * 128 + r, :],
                                  in_=sig[:r])
                for kt in range(NT_K):
                    cs = slice(kt * 128, (kt + 1) * 128)
                    pt = psT.tile([128, 128], f16, tag="tp")
                    nc.tensor.transpose(pt[:, :r], xh[:r, cs],
                                        ident[:r, :r])
                    nc.scalar.copy(out=xhatT[kt][:, i * 128:i * 128 + r],
                                   in_=pt[:, :r])
            mu_bc = persist.tile([128, LO], f32, tag="mu_bc")
            sig_bc = persist.tile([128, LO], f32, tag="sig_bc")
            own0 = WARM + K - 1
            for (dst, srcd) in ((mu_bc, mu_d), (sig_bc, sig_d)):
                s_ap = srcd[own0:own0 + LO, :]
                nc.sync.dma_start(
                    out=dst,
                    in_=bass.AP(tensor=s_ap.tensor, offset=s_ap.offset,
                                ap=[[0, 128], [1, LO]]))
            dres_cm.__exit__(None, None, None)
            p0_cm.__exit__(None, None, None)

            # ---- Phase A (+A2+B interleaved per d-tile) ----
            # mm1 + causal depthwise conv + silu -> X_main; gate mm2; and
            # the B/C projection accumulates incrementally so phase C can
            # start right after the last X_main tile.
            wbt = wstream.tile([128, NT_D, 2 * N], f16, tag="wbc")
            nc.sync.dma_start(
                out=wbt.rearrange("p kt e -> p (kt e)"),
                in_=WbcP_d[:, :])
            psbc = psB.tile([2 * N, LW], f32, tag="bc")
            X_main = []
            X_gate = []
            gate_silus = []
            for dt in range(NT_D if "A" in phases else 0):
                w1t = wstream.tile([128, NT_K, 128], f16, tag="w1")
                nc.sync.dma_start(
                    out=w1t.rearrange("p kt m -> p (kt m)"),
                    in_=W1P_d[dt * 128:(dt + 1) * 128, :])
                ps = psA.tile([128, LC], f32, tag="mm")
                for kt in range(NT_K):
                    nc.tensor.matmul(ps, w1t[:, kt, :],
                                     xhatT[kt],
                                     start=(kt == 0), stop=(kt == NT_K - 1))
                # depthwise conv: per-tap scaled copies on ACT (scale is the
                # per-channel tap weight), tap-sum via identity matmuls on PE
                # (keeps the tap accumulation off the bottleneck DVE)
                pre16 = work.tile([128, K, LC], f16, tag="pre")
                for tap in range(K):
                    nc.scalar.activation(out=pre16[:, tap, :], in_=ps,
                                         func=AF.Identity,
                                         bias=0.0,
                                         scale=convw_sb[dt][:, tap:tap + 1])
                psC = psA.tile([128, LW], f32, tag="mm")
                for tap in range(K):
                    nc.tensor.matmul(psC, ident,
                                     pre16[:, tap, tap:tap + LW],
                                     start=(tap == 0), stop=(tap == K - 1))
                xm = persist.tile([128, LW], f16, tag=f"xm{dt}")
                si = nc.scalar.activation(out=xm, in_=psC, func=AF.Silu,
                                          bias=cb2_sb[dt][:, 0:1], scale=1.0)
                gate_silus.append(si)
                X_main.append(xm)
                # incremental B/C projection accumulate
                nc.tensor.matmul(psbc, wbt[:, dt, :], xm,
                                 start=(dt == 0), stop=(dt == NT_D - 1))

            # ---- Phase B: bias/scale/mask + partition-broadcast ----
            bc_raw = work.tile([2 * N, LW], f32, tag="bcraw")
            nc.scalar.activation(out=bc_raw, in_=psbc, func=AF.Identity,
                                 bias=bbc_sb[:, 0:1], scale=1.0)
            bci = work.tile([2 * N, LW], f16, tag="bci")
            nc.vector.scalar_tensor_tensor(out=bci, in0=bc_raw,
                                           scalar=invAv_sb[:, 0:1],
                                           in1=mask_sb, op0=OP.mult,
                                           op1=OP.mult)
            Bm_bcI = persist.tile([128, N, LW], f16, tag="BmbcI")
            Cm_bc = persist.tile([128, N, LO], f16, tag="Cmbc")
            with tc.tile_pool(name="dstage", bufs=1, space="DRAM") as dpool:
                bci_dram = dpool.tile([2 * N, LW], f16, tag="bcid")
                nc.sync.dma_start(out=bci_dram, in_=bci)
                for n in range(N):
                    src_b = bci_dram[n:n + 1, :]
                    nc.sync.dma_start(
                        out=Bm_bcI[:, n, :],
                        in_=bass.AP(tensor=src_b.tensor, offset=src_b.offset,
                                    ap=[[0, 128]] + src_b.ap[1:]))
                    src_c = bci_dram[N + n:N + n + 1, WARM:LW]
                    nc.sync.dma_start(
                        out=Cm_bc[:, n, :],
                        in_=bass.AP(tensor=src_c.tensor, offset=src_c.offset,
                                    ap=[[0, 128]] + src_c.ap[1:]))

            # residual precompute: xres = xhatT*sig + mu + b_down (per e8)
            xres = []
            for e8 in range(NT_K):
                xrec = work.tile([128, LO], f32, tag="xrec")
                nc.gpsimd.tensor_tensor(out=xrec,
                                        in0=xhatT[e8]
                                        [:, WARM + K - 1:WARM + K - 1 + LO],
                                        in1=sig_bc, op=OP.mult)
                xr2 = persist.tile([128, LO], f32, tag=f"xres{e8}")
                nc.vector.scalar_tensor_tensor(
                    out=xr2, in0=xrec, scalar=bdown_sb[e8][:, 0:1],
                    in1=mu_bc, op0=OP.add, op1=OP.add)
                xres.append(xr2)

            # ---- Phase C: per d-tile: delta, a, u, scan, y ----
            # (phase D partial accumulations interleave after dt 9 and 13)
            y_gated = []
            xy1 = []

            def emit_phase_d_part(d0, d1, first):
                # accumulate sum_dt WdT.yg for dt in [d0,d1) into xy1[e8]
                for e8 in range(NT_K):
                    wdt = wstream.tile([128, d1 - d0, 128], f16,
                                       tag="wd8" if d1 - d0 == 4 else "wd0")
                    nc.sync.dma_start(
                        out=wdt.rearrange("p dt m -> p (dt m)"),
                        in_=WdP_d[e8 * 128:(e8 + 1) * 128,
                                  d0 * 128:d1 * 128])
                    ps = psD.tile([128, LO], f32, tag="mm")
                    for i in range(d1 - d0):
                        nc.tensor.matmul(ps, wdt[:, i, :], y_gated[d0 + i],
                                         start=(i == 0), stop=(i == d1 - d0 - 1))
                    xy = work.tile([128, LO], f32, tag="xy")
                    nc.scalar.activation(out=xy, in_=ps, func=AF.Identity,
                                         bias=0.0, scale=1.0)
                    if first:
                        xy2 = persist.tile([128, LO], f32, tag=f"xy2_{e8}")
                        nc.gpsimd.tensor_tensor(out=xy2, in0=xy,
                                                in1=xres[e8], op=OP.add)
                        xy1.append(xy2)
                    else:
                        nc.gpsimd.tensor_tensor(out=xy1[e8], in0=xy,
                                                in1=xy1[e8], op=OP.add)

            for dt in range(NT_D):
                # gate mm2 + silu built from the exp table:
                # silu(x) = x / (1 + exp(-x))  (avoids an ACT table switch)
                w2t = wstream.tile([128, NT_K, 128], f16, tag="w2")
                nc.sync.dma_start(
                    out=w2t.rearrange("p kt m -> p (kt m)"),
                    in_=W2P_d[dt * 128:(dt + 1) * 128, :])
                ps2 = psA.tile([128, LO], f32, tag="mm")
                for kt in range(NT_K):
                    nc.tensor.matmul(ps2, w2t[:, kt, :],
                                     xhatT[kt][:, WARM + K - 1:WARM + K - 1 + LO],
                                     start=(kt == 0), stop=(kt == NT_K - 1))
                eg = work.tile([128, LO], f16, tag="eg")
                nc.scalar.activation(out=eg, in_=ps2, func=AF.Exp,
                                     bias=nc2_sb[dt][:, 0:1], scale=-1.0)
                rg = work.tile([128, LO], f16, tag="rg")
                nc.vector.tensor_scalar(out=rg, in0=eg, scalar1=1.0,
                                        scalar2=None, op0=OP.add)
                with nc.allow_low_precision("sigmoid denominator, fp16 ok"):
                    nc.vector.reciprocal(out=rg, in_=rg)
                xg = persist.tile([128, LO], f16, tag=f"xg{dt}")
                nc.vector.scalar_tensor_tensor(
                    out=xg, in0=ps2, scalar=c2_sb[dt][:, 0:1], in1=rg,
                    op0=OP.add, op1=OP.mult)
                X_gate.append(xg)

                wllt = wstream.tile([128, NT_D, 128], f16, tag="wst")
                nc.sync.dma_start(
                    out=wllt.rearrange("p kt m -> p (kt m)"),
                    in_=WllP_d[dt * 128:(dt + 1) * 128, :])
                ps = psA.tile([128, LW], f32, tag="mm")
                for kt in range(NT_D):
                    nc.tensor.matmul(ps, wllt[:, kt, :],
                                     X_main[kt],
                                     start=(kt == 0), stop=(kt == NT_D - 1))
                # softplus(x) = ln(exp(x) + 1); exp & ln share one ACT table set
                e1 = work.tile([128, LW], f32, tag="e1")
                e1i = nc.scalar.activation(out=e1, in_=ps, func=AF.Exp,
                                           bias=bd_sb[dt][:, 0:1], scale=1.0)
                if dt == 0:
                    from concourse.tile_rust import add_dep_helper
                    for si in gate_silus:
                        add_dep_helper(e1i.ins, si.ins, False,
                                       "ACT table-set phase ordering")
                delta = work.tile([128, LW], f32, tag="delta")
                nc.scalar.activation(out=delta, in_=e1, func=AF.Ln,
                                     bias=1.0, scale=1.0)

                hc_eng = nc.vector if dt in HC_DVE else nc.gpsimd
                hci = red.tile([128, N, LO], f16, tag="hci")
                for g in range(NG):
                    ns = slice(g * NH, (g + 1) * NH)
                    a_t = big.tile([128, NH, LW], f16, tag=f"a{g}")
                    for i in range(NH):
                        n = g * NH + i
                        nc.scalar.activation(out=a_t[:, i, :], in_=delta,
                                             func=AF.Exp, bias=0.0,
                                             scale=A_sb[dt][:, n:n + 1])
                    w_t = bigwu.tile([128, NH, LW], f16, tag=f"w{g}")
                    nc.vector.tensor_tensor(
                        out=w_t, in0=bcast_n(X_main[dt], NH),
                        in1=Bm_bcI[:, ns, :], op=OP.mult)
                    # u = (a-1)*w as TS(4x) + in-place TT(2x): beats one STT
                    u_t = bigwu.tile([128, NH, LW], f16, tag=f"u{g}")
                    nc.vector.tensor_scalar(
                        out=u_t.rearrange("p n l -> p (n l)"),
                        in0=a_t.rearrange("p n l -> p (n l)"),
                        scalar1=-1.0, scalar2=None, op0=OP.add)
                    nc.vector.tensor_tensor(
                        out=u_t, in0=u_t, in1=w_t, op=OP.mult)
                    # zero decay at each n-segment start: encodes h(start)=u
                    nc.vector.memset(a_t[:, :, 0:1], 0.0)
                    # scan writes over w_t (dead once u is formed)
                    h_t = w_t
                    nc.vector.tensor_tensor_scan(
                        out=h_t.rearrange("p n l -> p (n l)"),
                        data0=a_t.rearrange("p n l -> p (n l)"),
                        data1=u_t.rearrange("p n l -> p (n l)"),
                        initial=0.0, op0=OP.mult, op1=OP.add)
                    hc_eng.tensor_tensor(
                        out=hci[:, ns, :],
                        in0=h_t[:, :, WARM:LW], in1=Cm_bc[:, ns, :],
                        op=OP.mult)
                # reduce over n: pairwise tree, in place in the low half
                # of hci (fp16 is 2x on DVE)
                for lv in (2, 4, 8):
                    hc_eng.tensor_tensor(out=hci[:, 0:N // lv, :],
                                         in0=hci[:, 0:N // lv, :],
                                         in1=hci[:, N // lv:2 * N // lv, :],
                                         op=OP.add)
                ysum = red.tile([128, LO], f16, tag="ysum")
                hc_eng.tensor_tensor(out=ysum, in0=hci[:, 0, :],
                                     in1=hci[:, 1, :], op=OP.add)
                yg = persist.tile([128, LO], f16, tag=f"yg{dt}")
                hc_eng.tensor_tensor(out=yg, in0=ysum, in1=X_gate[dt],
                                     op=OP.mult)
                y_gated.append(yg)
                if dt == 9:
                    emit_phase_d_part(0, 8, first=True)
                elif dt == 13:
                    emit_phase_d_part(8, 12, first=False)

            # ---- Phase D (last 4 d-tiles): down projection + residual ----
            for e8 in range(NT_K):
                wdt = wstream.tile([128, 4, 128], f16, tag="wd8")
                nc.sync.dma_start(
                    out=wdt.rearrange("p dt m -> p (dt m)"),
                    in_=WdP_d[e8 * 128:(e8 + 1) * 128, 12 * 128:])
                ps = psD.tile([128, LO], f32, tag="mm")
                for i in range(4):
                    nc.tensor.matmul(ps, wdt[:, i, :],
                                     y_gated[12 + i],
                                     start=(i == 0), stop=(i == 3))
                osb = work.tile([128, LO], f32, tag="osb")
                nc.vector.tensor_tensor(out=osb, in0=ps, in1=xy1[e8],
                                        op=OP.add)
                nc.sync.dma_start(out=Y_d[e8 * 128:(e8 + 1) * 128, :], in_=osb)

    nc.compile()
    return nc


def kernel(X, ln_g, ln_b, W_up1, conv_w, conv_b, W_ll, b_ll, A_log, W_up2,
           W_down, b_down):
    from concourse.bass_utils import run_bass_kernel_spmd

    f = np.float32
    h = np.float16
    X = np.asarray(X, f)
    A = -np.exp(np.asarray(A_log, f))
    assert np.allclose(A, A[0:1, :]), "kernel assumes A rows identical"
    c1 = (np.asarray(W_up1, f) @ np.asarray(ln_b, f)).astype(f)
    c2 = (np.asarray(W_up2, f) @ np.asarray(ln_b, f)).astype(f)
    cw = np.asarray(conv_w, f)[:, 0, :]                      # [D, K]
    cb2 = (np.asarray(conv_b, f) + c1 * cw.sum(1)).astype(f)
    # packed per-d-tile constants [128, NT_D, K+4+N]
    CW = K + 4 + N
    bd_ll = np.asarray(b_ll, f)
    cpk = np.empty((128, D // 128, CW), f)
    for dt in range(D // 128):
        r = slice(dt * 128, (dt + 1) * 128)
        cpk[:, dt, 0:K] = cw[r]
        cpk[:, dt, K] = cb2[r]
        cpk[:, dt, K + 1] = bd_ll[:D][r]
        cpk[:, dt, K + 2] = c2[r]
        cpk[:, dt, K + 3] = -c2[r]
        cpk[:, dt, K + 4:] = A[r]
    bvk = np.stack([bd_ll[D:],
                    np.concatenate([1.0 / A[0], np.ones(N, f)])], axis=1)
    def block(w, nt_out):
        # [C, E] -> [E//128*128, C] blocked: out[e_t*128+p, c_t*128+m]
        # = w[c_t*128+p, e_t*128+m]  (per-e-tile contiguous weight stream)
        C, E = w.shape
        return np.ascontiguousarray(
            w.reshape(C // 128, 128, E // 128, 128)
             .transpose(2, 1, 0, 3).reshape(E, C))

    W1T = (np.asarray(W_up1, f) * np.asarray(ln_g, f)[None, :]).T.astype(h)
    W2T = (np.asarray(W_up2, f) * np.asarray(ln_g, f)[None, :]).T.astype(h)
    WllT = np.asarray(W_ll, f).T.astype(h)
    WdT = np.asarray(W_down, f).T.astype(h)
    shared = {
        "W1P": block(W1T, D // 128),
        "W2P": block(W2T, D // 128),
        "WllP": block(WllT[:, :D], D // 128),
        "WbcP": np.ascontiguousarray(
            WllT[:, D:].reshape(D // 128, 128, 2 * N)
            .transpose(1, 0, 2).reshape(128, -1)),
        "WdP": block(WdT, D_OUTER // 128),
        "cpk": np.ascontiguousarray(cpk.reshape(128, -1)),
        "bvk": np.ascontiguousarray(bvk.astype(f)),
        "bdown": np.ascontiguousarray(
            np.asarray(b_down, f).reshape(D_OUTER // 128, 128).T),
    }
    in_maps = []
    for c in range(NCORES):
        b, q = divmod(c, 4)
        l0 = q * LO
        lo_ext = l0 - (WARM + K - 1)
        xs = np.zeros((LC, D_OUTER), f)
        src0 = max(0, lo_ext)
        hi = min(l0 + LO + 1, L)
        xs[src0 - lo_ext:src0 - lo_ext + (hi - src0), :] = X[b, src0:hi, :]
        mask = np.ones((1, LW), f)
        if q == 0:
            mask[0, :WARM] = 0.0
        in_maps.append({"Xs": xs, "mask": mask, **shared})

    nc = _build_program()
    res = run_bass_kernel_spmd(nc, in_maps, core_ids=list(range(NCORES)))
    global last_result
    last_result = res

    out = np.empty((B_SZ, L, D_OUTER), f)
    for c in range(NCORES):
        b, q = divmod(c, 4)
        out[b, q * LO:(q + 1) * LO, :] = res.results[c]["Y"].T
    return out


# revision 67
# speedup vs baseline: 1.2243x; 1.0058x over previous
"""Trainium2 Bass kernel for a Mamba-1-style MixerBlock.

Reference computation (shapes: X[2,1024,1024], D=2048, N=16, K=4):
  Xn = LayerNorm(X) * g + b
  X_main = silu(conv_b + causal_depthwise_conv1d(Xn @ W_up1.T))
  pp = X_main @ W_ll.T + b_ll ; delta = softplus(pp[:, :D]); Bm, Cm = pp[:, D:D+N], pp[:, D+N:]
  a = exp(delta * A)  (A = -exp(A_log), [D,N])
  u = (a-1)/A * Bm * X_main        (per (b,l,d,n))
  h[t] = a[t] h[t-1] + u[t]        (scan over L per (b,d,n))
  y_ssm[t,d] = sum_n Cm[t,n] h[t,d,n]
  out = X + (y_ssm * silu(Xn @ W_up2.T)) @ W_down.T + b_down

Sharding: sequence-parallel over 8 cores (2 batches x 4 L-quarters of 256).
Each core redundantly recomputes a WARM-step scan warmup (decays are fast),
so the kernel is embarrassingly parallel - no collectives.

Per-core layout: channels on partitions, sequence on the free dim.
All matmuls run in fp16 (PE 1 cycle/row; fp32 PSUM accumulate); the SSM
elementwise chain runs in fp16 (DVE 2x for tensor_tensor; scan keeps an
fp32 internal state). The L-scan is a native tensor_tensor_scan chaining
n-segments per instruction (decay zeroed at segment starts encodes h=u).
Engine balance: scan on POOL, u split DVE/POOL, w/hci/tree/gate on DVE.
"""

import functools
import numpy as np

D_OUTER, D, N, K = 1024, 2048, 16, 4
B_SZ, L = 2, 1024
NCORES = 8
LO = 256            # own sequence steps per core
WARM = 24           # redundant scan warmup steps
LW = WARM + LO      # domain of X_main/delta/scan
LC = LW + K         # LayerNorm/mm1 domain (conv taps + even pad)
NT_D = D // 128     # 16 d-tiles
NT_K = D_OUTER // 128  # 8 k-tiles over d_outer
last_result = None
NG = 2              # n-groups for a/w/u/scan (pipelining granularity)
NH = N // NG        # n-values per group
# d-tiles whose hci/tree/gate run on POOL (engine balance; scan/u are
# DVE-only: the Pool engine does not implement TensorScalarPtr)
HC_DVE = {0, 5, 9, 14, 15}


@functools.lru_cache(maxsize=2)
def _build_program(phases: str = "0ABCD"):
    import concourse.bass as bass
    import concourse.bacc as bacc
    import concourse.mybir as mybir
    import concourse.tile as tile
    from concourse.masks import make_identity

    f32 = mybir.dt.float32
    f16 = mybir.dt.float16
    AF = mybir.ActivationFunctionType
    OP = mybir.AluOpType

    # Steer the act-table-load pass: keep Exp and Ln only in their shared
    # set so phase C needs a single table load (ids/order preserved).
    import concourse.hw_specs as hw_specs
    if not getattr(bacc, "_act_tables_patched", False):
        _orig_gat = hw_specs.get_activation_tables

        def _gat(module_arch):
            tabs = _orig_gat(module_arch)
            AT = mybir.ActivationFunctionType
            for name, fns in tabs.items():
                if name != "natural_log_exp_and_others":
                    fns.discard(AT.Exp)
                    fns.discard(AT.Ln)
            return tabs

        bacc.get_activation_tables = _gat
        bacc._act_tables_patched = True

    nc = bacc.Bacc("TRN2", target_bir_lowering=False)

    # ---- DRAM I/O ----
    # Weights arrive pre-blocked so each per-d-tile stream is one contiguous
    # [128, contraction*128] read (2KB+ rows -> full DMA bandwidth).
    Xs_d = nc.dram_tensor("Xs", [LC, D_OUTER], f32, kind="ExternalInput")
    W1P_d = nc.dram_tensor("W1P", [NT_D * 128, NT_K * 128], f16,
                           kind="ExternalInput")
    W2P_d = nc.dram_tensor("W2P", [NT_D * 128, NT_K * 128], f16,
                           kind="ExternalInput")
    WllP_d = nc.dram_tensor("WllP", [NT_D * 128, NT_D * 128], f16,
                            kind="ExternalInput")
    WbcP_d = nc.dram_tensor("WbcP", [128, NT_D * 2 * N], f16,
                            kind="ExternalInput")
    WdP_d = nc.dram_tensor("WdP", [NT_K * 128, NT_D * 128], f16,
                           kind="ExternalInput")
    # packed per-d-tile constants: [128, NT_D, K+4+N]
    # (conv taps, conv bias, delta bias, gate bias, -gate bias, A row)
    CW = K + 4 + N
    cpk_d = nc.dram_tensor("cpk", [128, NT_D * CW], f32, kind="ExternalInput")
    # packed [2N, 2]: col 0 = b_ll[D:], col 1 = 1/A (B rows) or 1 (C rows)
    bvk_d = nc.dram_tensor("bvk", [2 * N, 2], f32, kind="ExternalInput")
    # packed [128, NT_K]: col e8 = b_down[e8*128:(e8+1)*128]
    bdown_d = nc.dram_tensor("bdown", [128, NT_K], f32, kind="ExternalInput")
    mask_d = nc.dram_tensor("mask", [1, LW], f32, kind="ExternalInput")
    Y_d = nc.dram_tensor("Y", [D_OUTER, LO], f32, kind="ExternalOutput")

    def bcast_n(t, nrep):
        # stride-0 broadcast of a [128, F] tile to [128, nrep, F]
        return bass.AP(tensor=t.tensor, offset=t.offset,
                       ap=[t.ap[0], [0, nrep], t.ap[1]])

    with tile.TileContext(nc) as tc:
        with (
            tc.tile_pool(name="const", bufs=1) as const,
            tc.tile_pool(name="persist", bufs=1) as persist,
            tc.tile_pool(name="work", bufs=2) as work,
            tc.tile_pool(name="big", bufs=2) as big,
            tc.tile_pool(name="bigwu", bufs=2) as bigwu,
            tc.tile_pool(name="red", bufs=2) as red,
            tc.tile_pool(name="wstream", bufs=2) as wstream,
            tc.tile_pool(name="psT", bufs=2, space="PSUM") as psT,
            tc.tile_pool(name="psB", bufs=1, space="PSUM") as psB,
            tc.tile_pool(name="psD", bufs=1, space="PSUM") as psD,
            tc.tile_pool(name="psA", bufs=4, space="PSUM") as psA,
        ):
            # ---- Phase 0 pool (row tiles processed one at a time) ----
            rows = [128, 128, LC - 256]
            p0_cm = tc.tile_pool(name="p0", bufs=2)
            p0 = p0_cm.__enter__()

            # ---- constants (packed DMAs) ----
            ident = const.tile([128, 128], f16, tag="ident")
            make_identity(nc, ident)
            eps_sb = const.tile([128, 1], f32, tag="eps")
            nc.vector.memset(eps_sb, 1e-5)

            cpk_sb = const.tile([128, NT_D, CW], f32, tag="cpk")
            nc.sync.dma_start(
                out=cpk_sb, in_=cpk_d.rearrange("p (dt f) -> p dt f", f=CW))
            convw_sb = [cpk_sb[:, dt, 0:K] for dt in range(NT_D)]
            cb2_sb = [cpk_sb[:, dt, K:K + 1] for dt in range(NT_D)]
            bd_sb = [cpk_sb[:, dt, K + 1:K + 2] for dt in range(NT_D)]
            c2_sb = [cpk_sb[:, dt, K + 2:K + 3] for dt in range(NT_D)]
            nc2_sb = [cpk_sb[:, dt, K + 3:K + 4] for dt in range(NT_D)]
            A_sb = [cpk_sb[:, dt, K + 4:K + 4 + N] for dt in range(NT_D)]

            bvk_sb = const.tile([2 * N, 2], f32, tag="bvk")
            nc.sync.dma_start(out=bvk_sb, in_=bvk_d[:, :])
            bbc_sb = bvk_sb[:, 0:1]
            invAv_sb = bvk_sb[:, 1:2]
            mask_sb = const.tile([2 * N, LW], f32, tag="mask")
            m_ap = mask_d[:, :]
            nc.sync.dma_start(
                out=mask_sb,
                in_=bass.AP(tensor=m_ap.tensor, offset=m_ap.offset,
                            ap=[[0, 2 * N], m_ap.ap[1]]))
            bdown_pk = const.tile([128, NT_K], f32, tag="bdn")
            nc.sync.dma_start(out=bdown_pk, in_=bdown_d[:, :])
            bdown_sb = [bdown_pk[:, e8:e8 + 1] for e8 in range(NT_K)]

            # ---- Phase 0: per row-tile: load, LayerNorm, transpose ----
            xhatT = []
            for kt in range(NT_K):
                xt = persist.tile([128, LC], f16, tag=f"xhT{kt}")
                xhatT.append(xt)
            dres_cm = tc.tile_pool(name="dres", bufs=1, space="DRAM")
            drp = dres_cm.__enter__()
            mu_d = drp.tile([3 * 128, 1], f32, tag="mu_d")
            sig_d = drp.tile([3 * 128, 1], f32, tag="sig_d")
            for i in range(3):
                r = rows[i]
                xr = p0.tile([128, D_OUTER], f32, tag="xr")
                # 4 chunked DMAs per row-tile to spread across DMA engines
                step = (r + 3) // 4
                for c0 in range(0, r, step):
                    c1 = min(c0 + step, r)
                    nc.sync.dma_start(
                        out=xr[c0:c1, :],
                        in_=Xs_d[i * 128 + c0:i * 128 + c1, :])
                # bn_stats free-dim max is 512: two subgroups then aggregate
                stats = work.tile([128, 2, 6], f32, tag="stats")
                for sg in range(2):
                    nc.vector.bn_stats(out=stats[:r, sg, :],
                                       in_=xr[:r, sg * 512:(sg + 1) * 512])
                mv = work.tile([128, 2], f32, tag="mv")
                nc.vector.bn_aggr(out=mv[:r, :], in_=stats[:r, :, :])
                sig = work.tile([128, 1], f32, tag="sig")
                nc.scalar.activation(out=sig[:r], in_=mv[:r, 1:2],
                                     func=AF.Sqrt, bias=eps_sb[:r, 0:1],
                                     scale=1.0)
                rsig = work.tile([128, 1], f32, tag="rsig")
                nc.vector.reciprocal(out=rsig[:r], in_=sig[:r])
                nmu = work.tile([128, 1], f32, tag="nmu")
                nc.vector.tensor_scalar(out=nmu[:r], in0=mv[:r, 0:1],
                                        scalar1=rsig[:r, 0:1], scalar2=-1.0,
                                        op0=OP.mult, op1=OP.mult)
                xh = p0.tile([128, D_OUTER], f16, tag="xh")
                nc.vector.tensor_scalar(out=xh[:r, :], in0=xr[:r, :],
                                        scalar1=rsig[:r, 0:1],
                                        scalar2=nmu[:r, 0:1],
                                        op0=OP.mult, op1=OP.add)
                # stage mu/sig to DRAM (read back broadcast for the residual)
                nc.sync.dma_start(out=mu_d[i * 128:i * 128 + r, :],
                                  in_=mv[:r, 0:1])
                nc.sync.dma_start(out=sig_d[i * 128:i * 128 + r, :],
                                  in_=sig[:r])
                for kt in range(NT_K):
                    cs = slice(kt * 128, (kt + 1) * 128)
                    pt = psT.tile([128, 128], f16, tag="tp")
                    nc.tensor.transpose(pt[:, :r], xh[:r, cs],
                                        ident[:r, :r])
                    nc.vector.tensor_copy(
                        out=xhatT[kt][:, i * 128:i * 128 + r],
                        in_=pt[:, :r])
            mu_bc = persist.tile([128, LO], f32, tag="mu_bc")
            sig_bc = persist.tile([128, LO], f32, tag="sig_bc")
            own0 = WARM + K - 1
            for (dst, srcd) in ((mu_bc, mu_d), (sig_bc, sig_d)):
                s_ap = srcd[own0:own0 + LO, :]
                nc.sync.dma_start(
                    out=dst,
                    in_=bass.AP(tensor=s_ap.tensor, offset=s_ap.offset,
                                ap=[[0, 128], [1, LO]]))
            dres_cm.__exit__(None, None, None)
            p0_cm.__exit__(None, None, None)

            # ---- Phase A (+A2+B interleaved per d-tile) ----
            # mm1 + causal depthwise conv + silu -> X_main; gate mm2; and
            # the B/C projection accumulates incrementally so phase C can
            # start right after the last X_main tile.
            wbt = wstream.tile([128, NT_D, 2 * N], f16, tag="wbc")
            nc.sync.dma_start(
                out=wbt.rearrange("p kt e -> p (kt e)"),
                in_=WbcP_d[:, :])
            psbc = psB.tile([2 * N, LW], f32, tag="bc")
            X_main = []
            X_gate = []
            gate_silus = []
            for dt in range(NT_D if "A" in phases else 0):
                w1t = wstream.tile([128, NT_K, 128], f16, tag="w1")
                nc.sync.dma_start(
                    out=w1t.rearrange("p kt m -> p (kt m)"),
                    in_=W1P_d[dt * 128:(dt + 1) * 128, :])
                ps = psA.tile([128, LC], f32, tag="mm")
                for kt in range(NT_K):
                    nc.tensor.matmul(ps, w1t[:, kt, :],
                                     xhatT[kt],
                                     start=(kt == 0), stop=(kt == NT_K - 1))
                # depthwise conv: per-tap scaled copies on DVE (independent
                # ops, phase A DVE is idle), tap-sum via ident matmuls on PE
                pre16 = work.tile([128, K, LC], f16, tag="pre")
                for tap in range(K):
                    nc.vector.tensor_scalar(
                        out=pre16[:, tap, :], in0=ps,
                        scalar1=convw_sb[dt][:, tap:tap + 1], scalar2=None,
                        op0=OP.mult)
                psC = psA.tile([128, LW], f32, tag="mm")
                for tap in range(K):
                    nc.tensor.matmul(psC, ident,
                                     pre16[:, tap, tap:tap + LW],
                                     start=(tap == 0), stop=(tap == K - 1))
                xm = persist.tile([128, LW], f16, tag=f"xm{dt}")
                si = nc.scalar.activation(out=xm, in_=psC, func=AF.Silu,
                                          bias=cb2_sb[dt][:, 0:1], scale=1.0)
                gate_silus.append(si)
                X_main.append(xm)
                # incremental B/C projection accumulate
                nc.tensor.matmul(psbc, wbt[:, dt, :], xm,
                                 start=(dt == 0), stop=(dt == NT_D - 1))

            # ---- Phase B: bias/scale/mask + partition-broadcast ----
            bc_raw = work.tile([2 * N, LW], f32, tag="bcraw")
            nc.scalar.activation(out=bc_raw, in_=psbc, func=AF.Identity,
                                 bias=bbc_sb[:, 0:1], scale=1.0)
            bci = work.tile([2 * N, LW], f16, tag="bci")
            nc.vector.scalar_tensor_tensor(out=bci, in0=bc_raw,
                                           scalar=invAv_sb[:, 0:1],
                                           in1=mask_sb, op0=OP.mult,
                                           op1=OP.mult)
            Bm_bcI = persist.tile([128, N, LW], f16, tag="BmbcI")
            Cm_bc = persist.tile([128, N, LO], f16, tag="Cmbc")
            with tc.tile_pool(name="dstage", bufs=1, space="DRAM") as dpool:
                bci_dram = dpool.tile([2 * N, LW], f16, tag="bcid")
                nc.sync.dma_start(out=bci_dram, in_=bci)
                for n in range(N):
                    src_b = bci_dram[n:n + 1, :]
                    nc.sync.dma_start(
                        out=Bm_bcI[:, n, :],
                        in_=bass.AP(tensor=src_b.tensor, offset=src_b.offset,
                                    ap=[[0, 128]] + src_b.ap[1:]))
                    src_c = bci_dram[N + n:N + n + 1, WARM:LW]
                    nc.sync.dma_start(
                        out=Cm_bc[:, n, :],
                        in_=bass.AP(tensor=src_c.tensor, offset=src_c.offset,
                                    ap=[[0, 128]] + src_c.ap[1:]))

            # residual precompute: xres = xhatT*sig + mu + b_down (per e8)
            xres = []
            for e8 in range(NT_K):
                xrec = work.tile([128, LO], f32, tag="xrec")
                nc.gpsimd.tensor_tensor(out=xrec,
                                        in0=xhatT[e8]
                                        [:, WARM + K - 1:WARM + K - 1 + LO],
                                        in1=sig_bc, op=OP.mult)
                xr2 = persist.tile([128, LO], f32, tag=f"xres{e8}")
                nc.vector.scalar_tensor_tensor(
                    out=xr2, in0=xrec, scalar=bdown_sb[e8][:, 0:1],
                    in1=mu_bc, op0=OP.add, op1=OP.add)
                xres.append(xr2)

            # ---- Phase C: per d-tile: delta, a, u, scan, y ----
            # (phase D partial accumulations interleave after dt 9 and 13)
            y_gated = []
            xy1 = []

            def emit_phase_d_part(d0, d1, first):
                # accumulate sum_dt WdT.yg for dt in [d0,d1) into xy1[e8]
                for e8 in range(NT_K):
                    wdt = wstream.tile([128, d1 - d0, 128], f16,
                                       tag="wd8" if d1 - d0 == 4 else "wd0")
                    nc.sync.dma_start(
                        out=wdt.rearrange("p dt m -> p (dt m)"),
                        in_=WdP_d[e8 * 128:(e8 + 1) * 128,
                                  d0 * 128:d1 * 128])
                    ps = psD.tile([128, LO], f32, tag="mm")
                    for i in range(d1 - d0):
                        nc.tensor.matmul(ps, wdt[:, i, :], y_gated[d0 + i],
                                         start=(i == 0), stop=(i == d1 - d0 - 1))
                    xy = work.tile([128, LO], f32, tag="xy")
                    nc.scalar.activation(out=xy, in_=ps, func=AF.Identity,
                                         bias=0.0, scale=1.0)
                    if first:
                        xy2 = persist.tile([128, LO], f32, tag=f"xy2_{e8}")
                        nc.gpsimd.tensor_tensor(out=xy2, in0=xy,
                                                in1=xres[e8], op=OP.add)
                        xy1.append(xy2)
                    else:
                        nc.gpsimd.tensor_tensor(out=xy1[e8], in0=xy,
                                                in1=xy1[e8], op=OP.add)

            for dt in range(NT_D):
                # gate mm2 + silu built from the exp table:
                # silu(x) = x / (1 + exp(-x))  (avoids an ACT table switch)
                w2t = wstream.tile([128, NT_K, 128], f16, tag="w2")
                nc.sync.dma_start(
                    out=w2t.rearrange("p kt m -> p (kt m)"),
                    in_=W2P_d[dt * 128:(dt + 1) * 128, :])
                ps2 = psA.tile([128, LO], f32, tag="mm")
                for kt in range(NT_K):
                    nc.tensor.matmul(ps2, w2t[:, kt, :],
                                     xhatT[kt][:, WARM + K - 1:WARM + K - 1 + LO],
                                     start=(kt == 0), stop=(kt == NT_K - 1))
                eg = work.tile([128, LO], f16, tag="eg")
                nc.scalar.activation(out=eg, in_=ps2, func=AF.Exp,
                                     bias=nc2_sb[dt][:, 0:1], scale=-1.0)
                rg = work.tile([128, LO], f16, tag="rg")
                nc.vector.tensor_scalar(out=rg, in0=eg, scalar1=1.0,
                                        scalar2=None, op0=OP.add)
                with nc.allow_low_precision("sigmoid denominator, fp16 ok"):
                    nc.vector.reciprocal(out=rg, in_=rg)
                xg = persist.tile([128, LO], f16, tag=f"xg{dt}")
                nc.vector.scalar_tensor_tensor(
                    out=xg, in0=ps2, scalar=c2_sb[dt][:, 0:1], in1=rg,
                    op0=OP.add, op1=OP.mult)
                X_gate.append(xg)

                wllt = wstream.tile([128, NT_D, 128], f16, tag="wst")
                nc.sync.dma_start(
                    out=wllt.rearrange("p kt m -> p (kt m)"),
                    in_=WllP_d[dt * 128:(dt + 1) * 128, :])
                ps = psA.tile([128, LW], f32, tag="mm")
                for kt in range(NT_D):
                    nc.tensor.matmul(ps, wllt[:, kt, :],
                                     X_main[kt],
                                     start=(kt == 0), stop=(kt == NT_D - 1))
                # softplus(x) = ln(exp(x) + 1); exp & ln share one ACT table set
                e1 = work.tile([128, LW], f32, tag="e1")
                e1i = nc.scalar.activation(out=e1, in_=ps, func=AF.Exp,
                                           bias=bd_sb[dt][:, 0:1], scale=1.0)
                if dt == 0:
                    from concourse.tile_rust import add_dep_helper
                    for si in gate_silus:
                        add_dep_helper(e1i.ins, si.ins, False,
                                       "ACT table-set phase ordering")
                delta = work.tile([128, LW], f32, tag="delta")
                nc.scalar.activation(out=delta, in_=e1, func=AF.Ln,
                                     bias=1.0, scale=1.0)

                hc_eng = nc.vector if dt in HC_DVE else nc.gpsimd
                hci = red.tile([128, N, LO], f16, tag="hci")
                for g in range(NG):
                    ns = slice(g * NH, (g + 1) * NH)
                    a_t = big.tile([128, NH, LW], f16, tag=f"a{g}")
                    for i in range(NH):
                        n = g * NH + i
                        nc.scalar.activation(out=a_t[:, i, :], in_=delta,
                                             func=AF.Exp, bias=0.0,
                                             scale=A_sb[dt][:, n:n + 1])
                    w_t = bigwu.tile([128, NH, LW], f16, tag=f"w{g}")
                    nc.vector.tensor_tensor(
                        out=w_t, in0=bcast_n(X_main[dt], NH),
                        in1=Bm_bcI[:, ns, :], op=OP.mult)
                    # u = (a-1)*w as TS(4x) + in-place TT(2x): beats one STT
                    u_t = bigwu.tile([128, NH, LW], f16, tag=f"u{g}")
                    nc.vector.tensor_scalar(
                        out=u_t.rearrange("p n l -> p (n l)"),
                        in0=a_t.rearrange("p n l -> p (n l)"),
                        scalar1=-1.0, scalar2=None, op0=OP.add)
                    nc.vector.tensor_tensor(
                        out=u_t, in0=u_t, in1=w_t, op=OP.mult)
                    # zero decay at each n-segment start: encodes h(start)=u
                    nc.vector.memset(a_t[:, :, 0:1], 0.0)
                    # scan writes over w_t (dead once u is formed)
                    h_t = w_t
                    nc.vector.tensor_tensor_scan(
                        out=h_t.rearrange("p n l -> p (n l)"),
                        data0=a_t.rearrange("p n l -> p (n l)"),
                        data1=u_t.rearrange("p n l -> p (n l)"),
                        initial=0.0, op0=OP.mult, op1=OP.add)
                    hc_eng.tensor_tensor(
                        out=hci[:, ns, :],
                        in0=h_t[:, :, WARM:LW], in1=Cm_bc[:, ns, :],
                        op=OP.mult)
                # reduce over n: pairwise tree, in place in the low half
                # of hci (fp16 is 2x on DVE)
                for lv in (2, 4, 8):
                    hc_eng.tensor_tensor(out=hci[:, 0:N // lv, :],
                                         in0=hci[:, 0:N // lv, :],
                                         in1=hci[:, N // lv:2 * N // lv, :],
                                         op=OP.add)
                ysum = red.tile([128, LO], f16, tag="ysum")
                hc_eng.tensor_tensor(out=ysum, in0=hci[:, 0, :],
                                     in1=hci[:, 1, :], op=OP.add)
                yg = persist.tile([128, LO], f16, tag=f"yg{dt}")
                hc_eng.tensor_tensor(out=yg, in0=ysum, in1=X_gate[dt],
                                     op=OP.mult)
                y_gated.append(yg)
                if dt == 9:
                    emit_phase_d_part(0, 8, first=True)
                elif dt == 13:
                    emit_phase_d_part(8, 12, first=False)

            # ---- Phase D (last 4 d-tiles): down projection + residual ----
            for e8 in range(NT_K):
                wdt = wstream.tile([128, 4, 128], f16, tag="wd8")
                nc.sync.dma_start(
                    out=wdt.rearrange("p dt m -> p (dt m)"),
                    in_=WdP_d[e8 * 128:(e8 + 1) * 128, 12 * 128:])
                ps = psD.tile([128, LO], f32, tag="mm")
                for i in range(4):
                    nc.tensor.matmul(ps, wdt[:, i, :],
                                     y_gated[12 + i],
                                     start=(i == 0), stop=(i == 3))
                osb = work.tile([128, LO], f32, tag="osb")
                nc.vector.tensor_tensor(out=osb, in0=ps, in1=xy1[e8],
                                        op=OP.add)
                nc.sync.dma_start(out=Y_d[e8 * 128:(e8 + 1) * 128, :], in_=osb)

    nc.compile()
    return nc


def kernel(X, ln_g, ln_b, W_up1, conv_w, conv_b, W_ll, b_ll, A_log, W_up2,
           W_down, b_down):
    from concourse.bass_utils import run_bass_kernel_spmd

    f = np.float32
    h = np.float16
    X = np.asarray(X, f)
    A = -np.exp(np.asarray(A_log, f))
    assert np.allclose(A, A[0:1, :]), "kernel assumes A rows identical"
    c1 = (np.asarray(W_up1, f) @ np.asarray(ln_b, f)).astype(f)
    c2 = (np.asarray(W_up2, f) @ np.asarray(ln_b, f)).astype(f)
    cw = np.asarray(conv_w, f)[:, 0, :]                      # [D, K]
    cb2 = (np.asarray(conv_b, f) + c1 * cw.sum(1)).astype(f)
    # packed per-d-tile constants [128, NT_D, K+4+N]
    CW = K + 4 + N
    bd_ll = np.asarray(b_ll, f)
    cpk = np.empty((128, D // 128, CW), f)
    for dt in range(D // 128):
        r = slice(dt * 128, (dt + 1) * 128)
        cpk[:, dt, 0:K] = cw[r]
        cpk[:, dt, K] = cb2[r]
        cpk[:, dt, K + 1] = bd_ll[:D][r]
        cpk[:, dt, K + 2] = c2[r]
        cpk[:, dt, K + 3] = -c2[r]
        cpk[:, dt, K + 4:] = A[r]
    bvk = np.stack([bd_ll[D:],
                    np.concatenate([1.0 / A[0], np.ones(N, f)])], axis=1)
    def block(w, nt_out):
        # [C, E] -> [E//128*128, C] blocked: out[e_t*128+p, c_t*128+m]
        # = w[c_t*128+p, e_t*128+m]  (per-e-tile contiguous weight stream)
        C, E = w.shape
        return np.ascontiguousarray(
            w.reshape(C // 128, 128, E // 128, 128)
             .transpose(2, 1, 0, 3).reshape(E, C))

    W1T = (np.asarray(W_up1, f) * np.asarray(ln_g, f)[None, :]).T.astype(h)
    W2T = (np.asarray(W_up2, f) * np.asarray(ln_g, f)[None, :]).T.astype(h)
    WllT = np.asarray(W_ll, f).T.astype(h)
    WdT = np.asarray(W_down, f).T.astype(h)
    shared = {
        "W1P": block(W1T, D // 128),
        "W2P": block(W2T, D // 128),
        "WllP": block(WllT[:, :D], D // 128),
        "WbcP": np.ascontiguousarray(
            WllT[:, D:].reshape(D // 128, 128, 2 * N)
            .transpose(1, 0, 2).reshape(128, -1)),
        "WdP": block(WdT, D_OUTER // 128),
        "cpk": np.ascontiguousarray(cpk.reshape(128, -1)),
        "bvk": np.ascontiguousarray(bvk.astype(f)),
        "bdown": np.ascontiguousarray(
            np.asarray(b_down, f).reshape(D_OUTER // 128, 128).T),
    }
    in_maps = []
    for c in range(NCORES):
        b, q = divmod(c, 4)
        l0 = q * LO
        lo_ext = l0 - (WARM + K - 1)
        xs = np.zeros((LC, D_OUTER), f)
        src0 = max(0, lo_ext)
        hi = min(l0 + LO + 1, L)
        xs[src0 - lo_ext:src0 - lo_ext + (hi - src0), :] = X[b, src0:hi, :]
        mask = np.ones((1, LW), f)
        if q == 0:
            mask[0, :WARM] = 0.0
        in_maps.append({"Xs": xs, "mask": mask, **shared})

    nc = _build_program()
    res = run_bass_kernel_spmd(nc, in_maps, core_ids=list(range(NCORES)))
    global last_result
    last_result = res

    out = np.empty((B_SZ, L, D_OUTER), f)
    for c in range(NCORES):
        b, q = divmod(c, 4)
        out[b, q * LO:(q + 1) * LO, :] = res.results[c]["Y"].T
    return out
